# revision 25
# baseline (speedup 1.0000x reference)
"""2-layer GCN (SpMM message passing) on 8 Trainium2 NeuronCores.

Strategy (row-sharded, gather-based):
  - Nodes are relabeled and assigned to (core, block, slot): 8 cores x BLOCKS
    blocks x 128 slots.  Assignment balances per-core edge counts and packs
    rows into blocks so every block's in-edge count fits CPB*128.
  - Each core computes Z = feat @ W1 for its own nodes (bf16), AllGathers the
    full Z into every core's HBM, then processes its own output rows:
    for each block, gather source rows Z[col] via dma_gather (bf16, 512B rows),
    build an S matrix (one-hot(slot) * edge_weight) on DVE, and accumulate
    H_block^T = sum_chunks msgs^T @ S on the TensorEngine in PSUM.
  - H^T feeds Y = H @ W2 directly (lhsT = H^T), Y is AllGathered, layer 2
    repeats the same gather/aggregate with the same edge data.
  - dma_gather has int16 indices, so each gather call addresses a <=32767-row
    window of the Z/Y buffer; windows are static per call (shared across
    cores), chosen from the data (edges sorted by source within a block).
"""

import os
import sys
import numpy as np

sys.path.insert(0, "/opt/trn_rl_repo")

from concourse import bass, bacc, mybir, tile  # noqa: E402

P = 128
F_IN = 256
F_HID = 256
F_OUT = 128
WIN = 32767  # dma_gather int16 index window
MAX_GCALL = int(os.environ.get("MAX_GCALL", "8"))  # chunks per gather call
# NOTE: one dma_gather call emits nch*128 descriptors into a per-queue SWDGE
# ring of dynamic_dma_scratch_size/16 = 1024 slots; >8 chunks/call hangs HW.
NQ = int(os.environ.get("NQ", "4"))  # SWDGE queues for gathers


def np_dt(dt):
    return mybir.dt.np(dt)


BF16 = np_dt(mybir.dt.bfloat16)


class Cfg:
    def __init__(self, n_nodes, n_edges, ncores, cpb, max_gcall=16):
        assert n_nodes % ncores == 0
        self.n = n_nodes
        self.e = n_edges
        self.ncores = ncores
        self.npc = n_nodes // ncores  # real nodes per core
        self.blocks = (self.npc + P - 1) // P
        if self.blocks * P - self.npc < 44:  # slack rows for bin packing
            self.blocks += 1
        self.cpb = cpb  # chunks (of 128 edges) per block
        self.max_gcall = max_gcall  # max chunks per dma_gather call
        self.totch = self.blocks * cpb  # total chunks per core
        self.bp = self.blocks * P  # padded nodes per core
        self.ntot = self.ncores * self.bp  # padded global nodes


def full_cfg():
    # capacity per core: blocks*cpb*128 must exceed max core edge count
    # 98 blocks * 32 chunks * 128 = 401408 > 400k + 2-3 sigma (assert-checked)
    return Cfg(100000, 3200000, 8, cpb=32, max_gcall=MAX_GCALL)


# --------------------------------------------------------------------------
# Host-side preprocessing
# --------------------------------------------------------------------------

def preprocess(cfg, feat, row, col, edge_weight, W1, b1, W2, b2):
    n, e = cfg.n, cfg.e
    deg_in = np.bincount(row, minlength=n)  # in-degree: edges aggregated per row

    # ---- assign nodes to cores (snake over degree-sorted nodes) ----
    order = np.argsort(-deg_in, kind="stable")
    ncores = cfg.ncores
    pos = np.arange(n)
    phase = pos % (2 * ncores)
    core_of_pos = np.where(phase < ncores, phase, 2 * ncores - 1 - phase)
    node_core = np.empty(n, dtype=np.int64)
    node_core[order] = core_of_pos

    # ---- within each core: snake nodes into blocks by degree ----
    node_block = np.empty(n, dtype=np.int64)
    node_slot = np.empty(n, dtype=np.int64)
    nb = cfg.blocks
    cap_edges = cfg.cpb * P
    for c in range(ncores):
        nodes = order[core_of_pos == c]  # degree-desc within core
        m = len(nodes)
        assert m == cfg.npc
        bpos = np.arange(m)
        ph = bpos % (2 * nb)
        blk = np.where(ph < nb, ph, 2 * nb - 1 - ph)
        # check block caps; fix overflows greedily
        cnt = np.bincount(blk, minlength=nb)
        esum = np.bincount(blk, weights=deg_in[nodes], minlength=nb)
        assert cnt.max() <= P, f"block row overflow {cnt.max()}"
        if esum.max() > cap_edges:
            blk = blk.copy()
            # move smallest-degree nodes out of overloaded blocks
            for b in np.where(esum > cap_edges)[0]:
                members = np.where(blk == b)[0]
                members = members[np.argsort(deg_in[nodes[members]])]
                k = 0
                while esum[b] > cap_edges and k < len(members):
                    mv = members[k]
                    d = deg_in[nodes[mv]]
                    cands = np.where(
                        (esum + d <= cap_edges) & (cnt < P))[0]
                    if len(cands) == 0:
                        raise RuntimeError("bin packing failed; raise cpb")
                    tgt = cands[np.argmin(esum[cands])]
                    blk[mv] = tgt
                    esum[b] -= d
                    esum[tgt] += d
                    cnt[b] -= 1
                    cnt[tgt] += 1
                    k += 1
            assert esum.max() <= cap_edges
        # slots within block
        slot = np.zeros(m, dtype=np.int64)
        so = np.argsort(blk, kind="stable")
        sb = blk[so]
        start = np.r_[0, np.flatnonzero(np.diff(sb)) + 1]
        sizes = np.diff(np.r_[start, m])
        ranks = np.arange(m) - np.repeat(start, sizes)
        slot[so] = ranks
        node_block[nodes] = blk
        node_slot[nodes] = slot

    newid = node_core * cfg.bp + node_block * P + node_slot

    # ---- edges: assign to (core, block) of their destination row ----
    er_new = newid[row]
    ec_new = newid[col]
    e_core = node_core[row]
    e_blk = node_block[row]
    e_slot = node_slot[row]
    gblk = e_core * cfg.blocks + e_blk  # global block id
    # serpentine: odd blocks sorted by descending col so gather calls can
    # span block boundaries within one int16 window
    sortcol = np.where(e_blk % 2 == 0, ec_new, cfg.ntot - 1 - ec_new)
    so = np.lexsort((sortcol, gblk))
    gblk_s = gblk[so]
    ec_s = ec_new[so]
    slot_s = e_slot[so]
    w_s = edge_weight[so]

    # per-core padded edge stream
    tot = cfg.totch * P
    nblk_g = ncores * cfg.blocks
    blk_cnt = np.bincount(gblk_s, minlength=nblk_g)
    assert blk_cnt.max() <= cap_edges, f"block edges {blk_cnt.max()} > {cap_edges}"
    blk_start = np.r_[0, np.cumsum(blk_cnt)[:-1]]
    rank_in_blk = np.arange(len(so)) - np.repeat(blk_start, blk_cnt)
    stream_pos = (gblk_s % cfg.blocks) * cap_edges + rank_in_blk  # within-core pos
    core_of_edge = gblk_s // cfg.blocks

    colpad = np.zeros((ncores, tot), dtype=np.int64)
    slotpad = np.zeros((ncores, tot), dtype=np.int64)
    wpad = np.zeros((ncores, tot), dtype=np.float32)
    validpad = np.zeros((ncores, tot), dtype=bool)
    colpad[core_of_edge, stream_pos] = ec_s
    slotpad[core_of_edge, stream_pos] = slot_s
    wpad[core_of_edge, stream_pos] = w_s
    validpad[core_of_edge, stream_pos] = True

    # ---- greedy gather-call plan (shared across cores) ----
    # Walk the chunk stream; pack consecutive chunks into one dma_gather call
    # while the cross-core col span of real edges stays within the int16
    # window and the call has at most max_gcall chunks.
    big = np.int64(1 << 60)
    nvc = validpad.reshape(ncores, cfg.totch, P)
    cvc = colpad.reshape(ncores, cfg.totch, P)
    ch_min = np.where(nvc, cvc, big).min(axis=(0, 2))  # per-chunk min col
    ch_max = np.where(nvc, cvc, -1).max(axis=(0, 2))
    call_start = []  # first chunk of each call
    call_len = []
    cur_lo, cur_hi = big, -1
    cur0, curn = 0, 0
    for t in range(cfg.totch):
        lo = min(cur_lo, ch_min[t])
        hi = max(cur_hi, ch_max[t])
        span_ok = (hi < 0) or (lo == big) or (hi - lo <= WIN - 1)
        if curn > 0 and (curn >= cfg.max_gcall or not span_ok):
            call_start.append(cur0)
            call_len.append(curn)
            cur0, curn = t, 0
            cur_lo, cur_hi = big, -1
            lo, hi = ch_min[t], ch_max[t]
        cur_lo, cur_hi = lo, hi
        curn += 1
    call_start.append(cur0)
    call_len.append(curn)
    calls = len(call_start)
    call_start = np.array(call_start)
    call_len = np.array(call_len)

    base = np.zeros(calls, dtype=np.int64)
    width = np.zeros(calls, dtype=np.int64)
    chunk_base = np.zeros(cfg.totch, dtype=np.int64)
    for t in range(calls):
        sl = slice(call_start[t], call_start[t] + call_len[t])
        lo = np.where(nvc[:, sl], cvc[:, sl], big).min()
        lo = 0 if lo == big else lo
        base[t] = min(lo, max(cfg.ntot - WIN, 0))
        width[t] = min(WIN, cfg.ntot - base[t])
        hi = np.where(nvc[:, sl], cvc[:, sl], -1).max()
        assert hi < base[t] + width[t], f"window overflow call {t}"
        chunk_base[sl] = base[t]

    # pad entries: idx = base of their call (always in-window), w = 0
    colpad = np.where(validpad, colpad,
                      np.broadcast_to(np.repeat(chunk_base, P), (ncores, tot)))
    wpad = np.where(validpad, wpad, 0.0)

    # ---- per-core device input planes ----
    idx16 = (colpad.reshape(ncores, cfg.totch, P)
             - np.repeat(chunk_base, P).reshape(cfg.totch, P)[None]).astype(np.int16)
    assert (idx16 >= 0).all()
    # idx plane: [128, totch*8]; per chunk t an [128, 8] block with
    # tile[p, t*8 + j] = idx[t, j*16 + p%16] -- calls read contiguous spans
    idxp = idx16.reshape(ncores, cfg.totch, 8, 16)
    idxp = idxp.transpose(0, 3, 1, 2).reshape(ncores, 16, cfg.totch * 8)
    idx_plane = np.tile(idxp, (1, 8, 1))  # replicate to 128 partitions

    # segid/w planes: [128, totch]; [p, t] = edge (t*128+p); fp32 scalars for
    # the per-chunk fused tensor_scalar (is_equal requires fp32 scalar)
    seg_plane = slotpad.reshape(ncores, cfg.totch, P).transpose(0, 2, 1)
    seg_plane = np.ascontiguousarray(seg_plane).astype(np.float32)
    w_plane = wpad.reshape(ncores, cfg.totch, P).transpose(0, 2, 1)
    w_plane = np.ascontiguousarray(w_plane).astype(np.float32)

    # iota row [128, 128] bf16: [p, s] = s (per-chunk S build via tensor_scalar)
    iota_plane = np.tile(np.arange(P).astype(BF16), (P, 1))

    # featT planes [2, 128, bp] bf16 per core
    feat_pad = np.zeros((cfg.ntot, F_IN), dtype=np.float32)
    feat_pad[newid] = feat
    featT = np.ascontiguousarray(
        feat_pad.reshape(ncores, cfg.bp, 2, P).transpose(0, 2, 3, 1)
    ).astype(BF16)  # [ncores, 2, 128, bp]

    w1p = np.ascontiguousarray(W1.reshape(2, P, F_HID)).astype(BF16)
    w2p = np.ascontiguousarray(W2.reshape(2, P, F_OUT)).astype(BF16)
    b1p = np.ascontiguousarray(b1.reshape(2, P, 1)).astype(np.float32)
    b2p = np.ascontiguousarray(b2.reshape(P, 1)).astype(np.float32)

    in_maps = []
    for c in range(ncores):
        in_maps.append({
            "featT": featT[c],
            "w1": w1p, "w2": w2p, "b1": b1p, "b2": b2p,
            "iota": iota_plane,
            "idxs": np.ascontiguousarray(idx_plane[c]),
            "segid": seg_plane[c],
            "wgt": w_plane[c],
        })
    meta = {
        "base": base.astype(np.int64),
        "width": width.astype(np.int64),
        "call_start": call_start,
        "call_len": call_len,
        "newid": newid,
        "node_core": node_core,
        "node_block": node_block,
        "node_slot": node_slot,
    }
    return in_maps, meta


def assemble(cfg, meta, outs):
    """outs: list per core of {'outT': [blocks,128,128] f32} -> [n, F_OUT]."""
    res = np.empty((cfg.n, F_OUT), dtype=np.float32)
    nc_, nb_, ns_ = meta["node_core"], meta["node_block"], meta["node_slot"]
    for c in range(cfg.ncores):
        o = outs[c]["outT"]  # [blocks, F_OUT, 128]
        sel = np.where(nc_ == c)[0]
        res[sel] = o[nb_[sel], :, ns_[sel]]
    return res


# --------------------------------------------------------------------------
# Device program
# --------------------------------------------------------------------------

def inner_bcast(ap, k):
    """Append a step-0 dim of size k to an AP (broadcast each element k times)."""
    return bass.AP(ap.tensor, ap.offset, list(ap.ap) + [[0, k]])


def outer_bcast(ap, k):
    """Insert a step-0 dim of size k before the last dim (repeat the last-dim
    sequence k times). Keeps the last dim contiguous for DVE 2x mode."""
    a = list(ap.ap)
    return bass.AP(ap.tensor, ap.offset, a[:-1] + [[0, k]] + a[-1:])


def strided_cols(ap, start, step, count):
    """Column view [p, start + step*j] of a contiguous [128, N] AP."""
    a = list(ap.ap)
    return bass.AP(ap.tensor, ap.offset + start, [a[0], [step, count]])


def build_program(tc, cfg, meta, outs, ins):
    nc = tc.nc
    dt = mybir.dt
    base, width = meta["base"], meta["width"]
    call_start, call_len = meta["call_start"], meta["call_len"]
    featT, w1, w2 = ins["featT"], ins["w1"], ins["w2"]
    b1, b2 = ins["b1"], ins["b2"]
    iota, idxs, segid, wgt = ins["iota"], ins["idxs"], ins["segid"], ins["wgt"]
    outT = outs["outT"]
    BL, CPB = cfg.blocks, cfg.cpb
    rg = [list(range(cfg.ncores))]

    with tc.tile_pool(name="const", bufs=1) as const, \
         tc.tile_pool(name="dram", bufs=1, space="DRAM") as dram, \
         tc.tile_pool(name="gpool", bufs=6) as gpool, \
         tc.tile_pool(name="spool", bufs=8) as spool, \
         tc.tile_pool(name="hpool", bufs=3) as hpool, \
         tc.tile_pool(name="psum_h", bufs=2, space="PSUM") as psum_h, \
         tc.tile_pool(name="psum_y", bufs=2, space="PSUM") as psum_y:
        # ------- constants -------
        idx_sb = const.tile([P, cfg.totch * 8], dt.int16)
        nc.sync.dma_start(idx_sb[:], idxs[:])
        seg_sb = const.tile([P, cfg.totch], dt.float32)
        nc.sync.dma_start(seg_sb[:], segid[:])
        w_sb = const.tile([P, cfg.totch], dt.float32)
        nc.sync.dma_start(w_sb[:], wgt[:])
        iota_sb = const.tile([P, P], dt.bfloat16)
        nc.sync.dma_start(iota_sb[:], iota[:])
        w1_sb = const.tile([P, 2 * F_HID], dt.bfloat16)
        nc.sync.dma_start(w1_sb[:, 0:F_HID], w1[0])
        nc.sync.dma_start(w1_sb[:, F_HID:2 * F_HID], w1[1])
        w2_sb = const.tile([P, 2 * F_OUT], dt.bfloat16)
        nc.sync.dma_start(w2_sb[:, 0:F_OUT], w2[0])
        nc.sync.dma_start(w2_sb[:, F_OUT:2 * F_OUT], w2[1])
        b1_sb = const.tile([P, 2], dt.float32)
        nc.sync.dma_start(b1_sb[:, 0:1], b1[0])
        nc.sync.dma_start(b1_sb[:, 1:2], b1[1])
        b2_sb = const.tile([P, 1], dt.float32)
        nc.sync.dma_start(b2_sb[:], b2[:])

        zdt = dt.bfloat16 if os.environ.get("NO_FP8") else dt.float8e4
        zin = dram.tile([cfg.bp, F_IN], zdt)
        zall = dram.tile([cfg.ntot, F_IN], zdt, addr_space="Shared")
        yin = dram.tile([cfg.bp, F_OUT], dt.bfloat16)
        yall = dram.tile([cfg.ntot, F_OUT], dt.bfloat16, addr_space="Shared")

        # ------- phase Z: Z = feat @ W1 for own shard -------
        with tc.tile_pool(name="zpool", bufs=3) as zpool, \
             tc.tile_pool(name="ftpool", bufs=1) as ftpool, \
             tc.tile_pool(name="psum_z", bufs=2, space="PSUM") as psum_z:
            ft_sb = ftpool.tile([P, 2 * cfg.bp], dt.bfloat16)
            nc.sync.dma_start(ft_sb[:, 0:cfg.bp], featT[0])
            nc.sync.dma_start(ft_sb[:, cfg.bp:2 * cfg.bp], featT[1])
            for g in range(BL):
                pz = psum_z.tile([P, F_HID], dt.float32, space="PSUM", tag="pz")
                nc.tensor.matmul(
                    out=pz[:], lhsT=ft_sb[:, g * P:(g + 1) * P],
                    rhs=w1_sb[:, 0:F_HID], start=True, stop=False)
                nc.tensor.matmul(
                    out=pz[:], lhsT=ft_sb[:, cfg.bp + g * P:cfg.bp + (g + 1) * P],
                    rhs=w1_sb[:, F_HID:2 * F_HID], start=False, stop=True)
                zb = zpool.tile([P, F_HID], zdt, tag="zb")
                nc.vector.tensor_copy(zb[:], pz[:])
                nc.sync.dma_start(zin[g * P:(g + 1) * P, :], zb[:])

        if cfg.ncores > 1 and not os.environ.get("SKIP_COLL"):
            nc.gpsimd.collective_compute(
                "AllGather", mybir.AluOpType.bypass, replica_groups=rg,
                ins=[zin.opt()], outs=[zall.opt()])
        else:
            nc.sync.dma_start(zall[0:cfg.bp, :], zin[:])

        # ------- layer 1 + Y -------
        def layer(src, felem, gdt, nhalves, out_cb):
            active = {}  # block -> ph list

            for t in range(len(call_start)):
                c0, nch = int(call_start[t]), int(call_len[t])
                gt = gpool.tile([P, cfg.max_gcall, felem], gdt,
                                tag="gt", name="gt")
                win = src[int(base[t]):int(base[t] + width[t]), :]
                nc.gpsimd.dma_gather(
                    gt[:, :nch, :], win, idx_sb[:, c0 * 8:(c0 + nch) * 8],
                    nch * P, nch * P, felem, elem_step=felem,
                    queue_num=(t % NQ))
                for cc in range(nch):
                    c = c0 + cc
                    b, cloc = c // CPB, c % CPB
                    if cloc == 0:
                        active[b] = [
                            psum_h.tile([P, P], dt.float32, space="PSUM",
                                        tag=f"ph{h}", name=f"ph{h}")
                            for h in range(nhalves)]
                    ph = active[b]
                    # S[p, s] = w[p,c] * (slot[p,c] == s): fused DVE 4x op,
                    # contiguous rhs for the matmul.
                    S = spool.tile([P, P], dt.bfloat16, tag="S", name="S")
                    nc.vector.tensor_scalar(
                        out=S[:], in0=iota_sb[:],
                        scalar1=seg_sb[:, c:c + 1], scalar2=w_sb[:, c:c + 1],
                        op0=mybir.AluOpType.is_equal,
                        op1=mybir.AluOpType.mult)
                    for h in range(nhalves):
                        nc.tensor.matmul(
                            out=ph[h][:],
                            lhsT=gt[:, cc, h * P:(h + 1) * P],
                            rhs=S[:],
                            start=(cloc == 0), stop=(cloc == CPB - 1))
                    if cloc == CPB - 1:
                        out_cb(b, ph)
                        del active[b]

        def l1_out(b, ph):
            hh = []
            for h in range(2):
                ht = hpool.tile([P, P], dt.bfloat16, tag=f"ht{h}")
                nc.scalar.activation(
                    ht[:], ph[h][:], mybir.ActivationFunctionType.Relu,
                    bias=b1_sb[:, h:h + 1])
                hh.append(ht)
            py = psum_y.tile([P, F_OUT], dt.float32, space="PSUM", tag="py")
            nc.tensor.matmul(out=py[:], lhsT=hh[0][:], rhs=w2_sb[:, 0:F_OUT],
                             start=True, stop=False)
            nc.tensor.matmul(out=py[:], lhsT=hh[1][:],
                             rhs=w2_sb[:, F_OUT:2 * F_OUT],
                             start=False, stop=True)
            yb = hpool.tile([P, F_OUT], dt.bfloat16, tag="yb")
            nc.vector.tensor_copy(yb[:], py[:])
            nc.sync.dma_start(yin[b * P:(b + 1) * P, :], yb[:])

        layer(zall, F_IN, zdt, 2, l1_out)

        if cfg.ncores > 1 and not os.environ.get("SKIP_COLL"):
            nc.gpsimd.collective_compute(
                "AllGather", mybir.AluOpType.bypass, replica_groups=rg,
                ins=[yin.opt()], outs=[yall.opt()])
        else:
            nc.sync.dma_start(yall[0:cfg.bp, :], yin[:])

        # ------- layer 2 -------
        def l2_out(b, ph):
            ob = hpool.tile([P, P], dt.float32, tag="ob")
            nc.scalar.activation(
                ob[:], ph[0][:], mybir.ActivationFunctionType.Identity,
                bias=b2_sb[:, 0:1])
            nc.sync.dma_start(outT[b], ob[:])

        layer(yall, F_OUT, dt.bfloat16, 1, l2_out)


# --------------------------------------------------------------------------
# Top level
# --------------------------------------------------------------------------

def declare_io(nc, cfg):
    dt = mybir.dt
    def di(name, shape, d):
        return nc.dram_tensor(name, shape, d, kind="ExternalInput").ap()
    ins = {
        "featT": di("featT", [2, P, cfg.bp], dt.bfloat16),
        "w1": di("w1", [2, P, F_HID], dt.bfloat16),
        "w2": di("w2", [2, P, F_OUT], dt.bfloat16),
        "b1": di("b1", [2, P, 1], dt.float32),
        "b2": di("b2", [P, 1], dt.float32),
        "iota": di("iota", [P, P], dt.bfloat16),
        "idxs": di("idxs", [P, cfg.totch * 8], dt.int16),
        "segid": di("segid", [P, cfg.totch], dt.float32),
        "wgt": di("wgt", [P, cfg.totch], dt.float32),
    }
    outs = {
        "outT": nc.dram_tensor("outT", [cfg.blocks, F_OUT, P], dt.float32,
                               kind="ExternalOutput").ap(),
    }
    return ins, outs


def build_nc(cfg, meta, repeat=1):
    scratch = int(os.environ.get("DMA_SCRATCH", "16384"))
    nc = bacc.Bacc("TRN2", target_bir_lowering=False, debug=False,
                   num_devices=cfg.ncores, num_swdge_queues=NQ,
                   dynamic_dma_scratch_size=scratch)
    ins, outs = declare_io(nc, cfg)
    with tile.TileContext(nc) as tc:
        for _ in range(repeat):
            build_program(tc, cfg, meta, outs, ins)
    nc.compile()
    return nc


# --------------------------------------------------------------------------
# Harness entry point: kernel(**inputs) with FULL unsharded inputs
# --------------------------------------------------------------------------

def kernel(feat, row, col, edge_weight, W1, b1, W2, b2):
    feat = np.asarray(feat, dtype=np.float32)
    row = np.asarray(row, dtype=np.int32)
    col = np.asarray(col, dtype=np.int32)
    edge_weight = np.asarray(edge_weight, dtype=np.float32)
    W1 = np.asarray(W1, dtype=np.float32)
    b1 = np.asarray(b1, dtype=np.float32)
    W2 = np.asarray(W2, dtype=np.float32)
    b2 = np.asarray(b2, dtype=np.float32)

    cfg = full_cfg()
    assert feat.shape == (cfg.n, F_IN) and row.shape == (cfg.e,)

    in_maps, meta = preprocess(cfg, feat, row, col, edge_weight, W1, b1, W2, b2)
    nc = build_nc(cfg, meta)

    from concourse.bass_utils import run_bass_kernel_spmd
    res = run_bass_kernel_spmd(nc, in_maps, core_ids=list(range(cfg.ncores)))
    outs = [{"outT": r["outT"]} for r in res.results]
    return assemble(cfg, meta, outs)



# revision 26
# speedup vs baseline: 3.6958x; 3.6958x over previous
"""2-layer GCN (SpMM message passing) on 8 Trainium2 NeuronCores.

Strategy (row-sharded, gather-based):
  - Nodes are relabeled and assigned to (core, block, slot): 8 cores x BLOCKS
    blocks x 128 slots.  Assignment balances per-core edge counts and packs
    rows into blocks so every block's in-edge count fits CPB*128.
  - Each core computes Z = feat @ W1 for its own nodes (bf16), AllGathers the
    full Z into every core's HBM, then processes its own output rows:
    for each block, gather source rows Z[col] via dma_gather (bf16, 512B rows),
    build an S matrix (one-hot(slot) * edge_weight) on DVE, and accumulate
    H_block^T = sum_chunks msgs^T @ S on the TensorEngine in PSUM.
  - H^T feeds Y = H @ W2 directly (lhsT = H^T), Y is AllGathered, layer 2
    repeats the same gather/aggregate with the same edge data.
  - dma_gather has int16 indices, so each gather call addresses a <=32767-row
    window of the Z/Y buffer; windows are static per call (shared across
    cores), chosen from the data (edges sorted by source within a block).
"""

import os
import sys
import numpy as np

sys.path.insert(0, "/opt/trn_rl_repo")

from concourse import bass, bacc, mybir, tile  # noqa: E402

P = 128
F_IN = 256
F_HID = 256
F_OUT = 128
WIN = 32767  # dma_gather int16 index window
MAX_GCALL = int(os.environ.get("MAX_GCALL", "8"))  # chunks per gather call
NQ = int(os.environ.get("NQ", "4"))  # SWDGE queues for gathers


def np_dt(dt):
    return mybir.dt.np(dt)


BF16 = np_dt(mybir.dt.bfloat16)


class Cfg:
    def __init__(self, n_nodes, n_edges, ncores, cpb, max_gcall=16):
        assert n_nodes % ncores == 0
        self.n = n_nodes
        self.e = n_edges
        self.ncores = ncores
        self.npc = n_nodes // ncores  # real nodes per core
        self.blocks = (self.npc + P - 1) // P
        if self.blocks * P - self.npc < 44:  # slack rows for bin packing
            self.blocks += 1
        self.cpb = cpb  # chunks (of 128 edges) per block
        self.max_gcall = max_gcall  # max chunks per dma_gather call
        self.totch = self.blocks * cpb  # total chunks per core
        self.bp = self.blocks * P  # padded nodes per core
        self.ntot = self.ncores * self.bp  # padded global nodes


def full_cfg():
    # capacity per core: blocks*cpb*128 must exceed max core edge count
    # 98 blocks * 32 chunks * 128 = 401408 > 400k + 2-3 sigma (assert-checked)
    return Cfg(100000, 3200000, 8, cpb=32, max_gcall=MAX_GCALL)


# --------------------------------------------------------------------------
# Host-side preprocessing
# --------------------------------------------------------------------------

def preprocess(cfg, feat, row, col, edge_weight, W1, b1, W2, b2):
    n, e = cfg.n, cfg.e
    deg_in = np.bincount(row, minlength=n)  # in-degree: edges aggregated per row

    # ---- assign nodes to cores (snake over degree-sorted nodes) ----
    order = np.argsort(-deg_in, kind="stable")
    ncores = cfg.ncores
    pos = np.arange(n)
    phase = pos % (2 * ncores)
    core_of_pos = np.where(phase < ncores, phase, 2 * ncores - 1 - phase)
    node_core = np.empty(n, dtype=np.int64)
    node_core[order] = core_of_pos

    # ---- within each core: snake nodes into blocks by degree ----
    node_block = np.empty(n, dtype=np.int64)
    node_slot = np.empty(n, dtype=np.int64)
    nb = cfg.blocks
    cap_edges = cfg.cpb * P
    for c in range(ncores):
        nodes = order[core_of_pos == c]  # degree-desc within core
        m = len(nodes)
        assert m == cfg.npc
        bpos = np.arange(m)
        ph = bpos % (2 * nb)
        blk = np.where(ph < nb, ph, 2 * nb - 1 - ph)
        # check block caps; fix overflows greedily
        cnt = np.bincount(blk, minlength=nb)
        esum = np.bincount(blk, weights=deg_in[nodes], minlength=nb)
        assert cnt.max() <= P, f"block row overflow {cnt.max()}"
        if esum.max() > cap_edges:
            blk = blk.copy()
            # move smallest-degree nodes out of overloaded blocks
            for b in np.where(esum > cap_edges)[0]:
                members = np.where(blk == b)[0]
                members = members[np.argsort(deg_in[nodes[members]])]
                k = 0
                while esum[b] > cap_edges and k < len(members):
                    mv = members[k]
                    d = deg_in[nodes[mv]]
                    cands = np.where(
                        (esum + d <= cap_edges) & (cnt < P))[0]
                    if len(cands) == 0:
                        raise RuntimeError("bin packing failed; raise cpb")
                    tgt = cands[np.argmin(esum[cands])]
                    blk[mv] = tgt
                    esum[b] -= d
                    esum[tgt] += d
                    cnt[b] -= 1
                    cnt[tgt] += 1
                    k += 1
            assert esum.max() <= cap_edges
        # slots within block
        slot = np.zeros(m, dtype=np.int64)
        so = np.argsort(blk, kind="stable")
        sb = blk[so]
        start = np.r_[0, np.flatnonzero(np.diff(sb)) + 1]
        sizes = np.diff(np.r_[start, m])
        ranks = np.arange(m) - np.repeat(start, sizes)
        slot[so] = ranks
        node_block[nodes] = blk
        node_slot[nodes] = slot

    newid = node_core * cfg.bp + node_block * P + node_slot

    # ---- edges: assign to (core, block) of their destination row ----
    er_new = newid[row]
    ec_new = newid[col]
    e_core = node_core[row]
    e_blk = node_block[row]
    e_slot = node_slot[row]
    gblk = e_core * cfg.blocks + e_blk  # global block id
    # serpentine: odd blocks sorted by descending col so gather calls can
    # span block boundaries within one int16 window
    sortcol = np.where(e_blk % 2 == 0, ec_new, cfg.ntot - 1 - ec_new)
    so = np.lexsort((sortcol, gblk))
    gblk_s = gblk[so]
    ec_s = ec_new[so]
    slot_s = e_slot[so]
    w_s = edge_weight[so]

    # per-core padded edge stream
    tot = cfg.totch * P
    nblk_g = ncores * cfg.blocks
    blk_cnt = np.bincount(gblk_s, minlength=nblk_g)
    assert blk_cnt.max() <= cap_edges, f"block edges {blk_cnt.max()} > {cap_edges}"
    blk_start = np.r_[0, np.cumsum(blk_cnt)[:-1]]
    rank_in_blk = np.arange(len(so)) - np.repeat(blk_start, blk_cnt)
    stream_pos = (gblk_s % cfg.blocks) * cap_edges + rank_in_blk  # within-core pos
    core_of_edge = gblk_s // cfg.blocks

    colpad = np.zeros((ncores, tot), dtype=np.int64)
    slotpad = np.zeros((ncores, tot), dtype=np.int64)
    wpad = np.zeros((ncores, tot), dtype=np.float32)
    validpad = np.zeros((ncores, tot), dtype=bool)
    colpad[core_of_edge, stream_pos] = ec_s
    slotpad[core_of_edge, stream_pos] = slot_s
    wpad[core_of_edge, stream_pos] = w_s
    validpad[core_of_edge, stream_pos] = True

    # ---- greedy gather-call plan (shared across cores) ----
    # Walk the chunk stream; pack consecutive chunks into one dma_gather call
    # while the cross-core col span of real edges stays within the int16
    # window and the call has at most max_gcall chunks.
    big = np.int64(1 << 60)
    nvc = validpad.reshape(ncores, cfg.totch, P)
    cvc = colpad.reshape(ncores, cfg.totch, P)
    ch_min = np.where(nvc, cvc, big).min(axis=(0, 2))  # per-chunk min col
    ch_max = np.where(nvc, cvc, -1).max(axis=(0, 2))
    call_start = []  # first chunk of each call
    call_len = []
    cur_lo, cur_hi = big, -1
    cur0, curn = 0, 0
    for t in range(cfg.totch):
        lo = min(cur_lo, ch_min[t])
        hi = max(cur_hi, ch_max[t])
        span_ok = (hi < 0) or (lo == big) or (hi - lo <= WIN - 1)
        if curn > 0 and (curn >= cfg.max_gcall or not span_ok):
            call_start.append(cur0)
            call_len.append(curn)
            cur0, curn = t, 0
            cur_lo, cur_hi = big, -1
            lo, hi = ch_min[t], ch_max[t]
        cur_lo, cur_hi = lo, hi
        curn += 1
    call_start.append(cur0)
    call_len.append(curn)
    calls = len(call_start)
    call_start = np.array(call_start)
    call_len = np.array(call_len)

    base = np.zeros(calls, dtype=np.int64)
    width = np.zeros(calls, dtype=np.int64)
    chunk_base = np.zeros(cfg.totch, dtype=np.int64)
    for t in range(calls):
        sl = slice(call_start[t], call_start[t] + call_len[t])
        lo = np.where(nvc[:, sl], cvc[:, sl], big).min()
        lo = 0 if lo == big else lo
        base[t] = min(lo, max(cfg.ntot - WIN, 0))
        width[t] = min(WIN, cfg.ntot - base[t])
        hi = np.where(nvc[:, sl], cvc[:, sl], -1).max()
        assert hi < base[t] + width[t], f"window overflow call {t}"
        chunk_base[sl] = base[t]

    # pad entries: idx = base of their call (always in-window), w = 0
    colpad = np.where(validpad, colpad,
                      np.broadcast_to(np.repeat(chunk_base, P), (ncores, tot)))
    wpad = np.where(validpad, wpad, 0.0)

    # ---- per-core device input planes ----
    idx16 = (colpad.reshape(ncores, cfg.totch, P)
             - np.repeat(chunk_base, P).reshape(cfg.totch, P)[None]).astype(np.int16)
    assert (idx16 >= 0).all()
    # idx plane: [128, totch*8]; per chunk t an [128, 8] block with
    # tile[p, t*8 + j] = idx[t, j*16 + p%16] -- calls read contiguous spans
    idxp = idx16.reshape(ncores, cfg.totch, 8, 16)
    idxp = idxp.transpose(0, 3, 1, 2).reshape(ncores, 16, cfg.totch * 8)
    idx_plane = np.tile(idxp, (1, 8, 1))  # replicate to 128 partitions

    # segid/w planes: [128, totch]; [p, t] = edge (t*128+p); bf16 (values 0..127
    # are exact) so the S-generation tensor_tensor qualifies for DVE 2x mode
    seg_plane = slotpad.reshape(ncores, cfg.totch, P).transpose(0, 2, 1)
    seg_plane = np.ascontiguousarray(seg_plane).astype(BF16)
    w_plane = wpad.reshape(ncores, cfg.totch, P).transpose(0, 2, 1)
    w_plane = np.ascontiguousarray(w_plane).astype(BF16)

    # iotaT plane [128, 128*cpb] bf16: [p, s*cpb + c] = s  (transposed S layout)
    iota_plane = np.repeat(np.arange(P), cfg.cpb).astype(BF16)
    iota_plane = np.tile(iota_plane, (P, 1))

    # featT planes [2, 128, bp] bf16 per core
    feat_pad = np.zeros((cfg.ntot, F_IN), dtype=np.float32)
    feat_pad[newid] = feat
    featT = np.ascontiguousarray(
        feat_pad.reshape(ncores, cfg.bp, 2, P).transpose(0, 2, 3, 1)
    ).astype(BF16)  # [ncores, 2, 128, bp]

    w1p = np.ascontiguousarray(W1.reshape(2, P, F_HID)).astype(BF16)
    w2p = np.ascontiguousarray(W2.reshape(2, P, F_OUT)).astype(BF16)
    b1p = np.ascontiguousarray(b1.reshape(2, P, 1)).astype(np.float32)
    b2p = np.ascontiguousarray(b2.reshape(P, 1)).astype(np.float32)

    in_maps = []
    for c in range(ncores):
        in_maps.append({
            "featT": featT[c],
            "w1": w1p, "w2": w2p, "b1": b1p, "b2": b2p,
            "iota": iota_plane,
            "idxs": np.ascontiguousarray(idx_plane[c]),
            "segid": seg_plane[c],
            "wgt": w_plane[c],
        })
    meta = {
        "base": base.astype(np.int64),
        "width": width.astype(np.int64),
        "call_start": call_start,
        "call_len": call_len,
        "newid": newid,
        "node_core": node_core,
        "node_block": node_block,
        "node_slot": node_slot,
    }
    return in_maps, meta


def assemble(cfg, meta, outs):
    """outs: list per core of {'outT': [blocks,128,128] f32} -> [n, F_OUT]."""
    res = np.empty((cfg.n, F_OUT), dtype=np.float32)
    nc_, nb_, ns_ = meta["node_core"], meta["node_block"], meta["node_slot"]
    for c in range(cfg.ncores):
        o = outs[c]["outT"]  # [blocks, F_OUT, 128]
        sel = np.where(nc_ == c)[0]
        res[sel] = o[nb_[sel], :, ns_[sel]]
    return res


# --------------------------------------------------------------------------
# Device program
# --------------------------------------------------------------------------

def inner_bcast(ap, k):
    """Append a step-0 dim of size k to an AP (broadcast each element k times)."""
    return bass.AP(ap.tensor, ap.offset, list(ap.ap) + [[0, k]])


def outer_bcast(ap, k):
    """Insert a step-0 dim of size k before the last dim (repeat the last-dim
    sequence k times). Keeps the last dim contiguous for DVE 2x mode."""
    a = list(ap.ap)
    return bass.AP(ap.tensor, ap.offset, a[:-1] + [[0, k]] + a[-1:])


def strided_cols(ap, start, step, count):
    """Column view [p, start + step*j] of a contiguous [128, N] AP."""
    a = list(ap.ap)
    return bass.AP(ap.tensor, ap.offset + start, [a[0], [step, count]])


def build_program(tc, cfg, meta, outs, ins):
    nc = tc.nc
    dt = mybir.dt
    base, width = meta["base"], meta["width"]
    call_start, call_len = meta["call_start"], meta["call_len"]
    featT, w1, w2 = ins["featT"], ins["w1"], ins["w2"]
    b1, b2 = ins["b1"], ins["b2"]
    iota, idxs, segid, wgt = ins["iota"], ins["idxs"], ins["segid"], ins["wgt"]
    outT = outs["outT"]
    BL, CPB = cfg.blocks, cfg.cpb
    rg = [list(range(cfg.ncores))]

    with tc.tile_pool(name="const", bufs=1) as const, \
         tc.tile_pool(name="dram", bufs=1, space="DRAM") as dram, \
         tc.tile_pool(name="gpool", bufs=6) as gpool, \
         tc.tile_pool(name="spool", bufs=2) as spool, \
         tc.tile_pool(name="hpool", bufs=3) as hpool, \
         tc.tile_pool(name="psum_h", bufs=2, space="PSUM") as psum_h, \
         tc.tile_pool(name="psum_y", bufs=2, space="PSUM") as psum_y:
        # ------- constants -------
        idx_sb = const.tile([P, cfg.totch * 8], dt.int16)
        nc.sync.dma_start(idx_sb[:], idxs[:])
        seg_sb = const.tile([P, cfg.totch], dt.bfloat16)
        nc.sync.dma_start(seg_sb[:], segid[:])
        w_sb = const.tile([P, cfg.totch], dt.bfloat16)
        nc.sync.dma_start(w_sb[:], wgt[:])
        iota_sb = const.tile([P, CPB * P], dt.bfloat16)
        nc.sync.dma_start(iota_sb[:], iota[:])
        w1_sb = const.tile([P, 2 * F_HID], dt.bfloat16)
        nc.sync.dma_start(w1_sb[:, 0:F_HID], w1[0])
        nc.sync.dma_start(w1_sb[:, F_HID:2 * F_HID], w1[1])
        w2_sb = const.tile([P, 2 * F_OUT], dt.bfloat16)
        nc.sync.dma_start(w2_sb[:, 0:F_OUT], w2[0])
        nc.sync.dma_start(w2_sb[:, F_OUT:2 * F_OUT], w2[1])
        b1_sb = const.tile([P, 2], dt.float32)
        nc.sync.dma_start(b1_sb[:, 0:1], b1[0])
        nc.sync.dma_start(b1_sb[:, 1:2], b1[1])
        b2_sb = const.tile([P, 1], dt.float32)
        nc.sync.dma_start(b2_sb[:], b2[:])

        zin = dram.tile([cfg.bp, F_IN], dt.bfloat16)
        zall = dram.tile([cfg.ntot, F_IN], dt.bfloat16, addr_space="Shared")
        yin = dram.tile([cfg.bp, F_OUT], dt.bfloat16)
        yall = dram.tile([cfg.ntot, F_OUT], dt.bfloat16, addr_space="Shared")

        # ------- phase Z: Z = feat @ W1 for own shard -------
        with tc.tile_pool(name="zpool", bufs=3) as zpool, \
             tc.tile_pool(name="ftpool", bufs=1) as ftpool, \
             tc.tile_pool(name="psum_z", bufs=2, space="PSUM") as psum_z:
            ft_sb = ftpool.tile([P, 2 * cfg.bp], dt.bfloat16)
            nc.sync.dma_start(ft_sb[:, 0:cfg.bp], featT[0])
            nc.sync.dma_start(ft_sb[:, cfg.bp:2 * cfg.bp], featT[1])
            for g in range(BL):
                pz = psum_z.tile([P, F_HID], dt.float32, space="PSUM", tag="pz")
                nc.tensor.matmul(
                    out=pz[:], lhsT=ft_sb[:, g * P:(g + 1) * P],
                    rhs=w1_sb[:, 0:F_HID], start=True, stop=False)
                nc.tensor.matmul(
                    out=pz[:], lhsT=ft_sb[:, cfg.bp + g * P:cfg.bp + (g + 1) * P],
                    rhs=w1_sb[:, F_HID:2 * F_HID], start=False, stop=True)
                zb = zpool.tile([P, F_HID], dt.bfloat16, tag="zb")
                nc.vector.tensor_copy(zb[:], pz[:])
                nc.sync.dma_start(zin[g * P:(g + 1) * P, :], zb[:])

        if cfg.ncores > 1 and not os.environ.get("SKIP_COLL"):
            nc.gpsimd.collective_compute(
                "AllGather", mybir.AluOpType.bypass, replica_groups=rg,
                ins=[zin.opt()], outs=[zall.opt()])
        else:
            nc.sync.dma_start(zall[0:cfg.bp, :], zin[:])

        # ------- layer 1 + Y -------
        def layer(src, felem, nhalves, out_cb):
            active = {}  # block -> (S, ph list)

            def start_block(b):
                # S in transposed layout: S[p, s*CPB + c] = w[p,c]*(slot==s).
                # All operands keep a contiguous last dim -> DVE 2x mode.
                eq = spool.tile([P, CPB * P], dt.bfloat16, tag="eq", name="eq")
                S = spool.tile([P, CPB * P], dt.bfloat16, tag="S", name="S")
                if os.environ.get("SKIP_SGEN"):
                    nc.vector.tensor_copy(S[:], iota_sb[:, 0:CPB * P])
                    active[b] = (S, [psum_h.tile(
                        [P, P], dt.float32, space="PSUM", tag=f"ph{h}",
                        name=f"ph{h}") for h in range(nhalves)])
                    return
                nc.vector.tensor_tensor(
                    out=eq[:], in0=iota_sb[:],
                    in1=outer_bcast(seg_sb[:, b * CPB:(b + 1) * CPB], P),
                    op=mybir.AluOpType.is_equal)
                nc.vector.tensor_tensor(
                    out=S[:], in0=eq[:],
                    in1=outer_bcast(w_sb[:, b * CPB:(b + 1) * CPB], P),
                    op=mybir.AluOpType.mult)
                ph = [psum_h.tile([P, P], dt.float32, space="PSUM",
                                  tag=f"ph{h}", name=f"ph{h}")
                      for h in range(nhalves)]
                active[b] = (S, ph)

            for t in range(len(call_start)):
                c0, nch = int(call_start[t]), int(call_len[t])
                gt = gpool.tile([P, cfg.max_gcall, felem], dt.bfloat16,
                                tag="gt", name="gt")
                win = src[int(base[t]):int(base[t] + width[t]), :]
                if os.environ.get("GATHER_HALF") and felem == 256:
                    # same calls/descriptors, half the bytes (junk numerics)
                    gh = gpool.tile([P, cfg.max_gcall, felem // 2],
                                    dt.bfloat16, tag="gh", name="gh")
                    nc.gpsimd.dma_gather(
                        gh[:, :nch, :], win, idx_sb[:, c0 * 8:(c0 + nch) * 8],
                        nch * P, nch * P, felem // 2, elem_step=felem,
                        queue_num=(t % NQ))
                    nc.vector.tensor_copy(gt[:, 0, 0:P], iota_sb[:, 0:P])
                else:
                    nc.gpsimd.dma_gather(
                        gt[:, :nch, :], win, idx_sb[:, c0 * 8:(c0 + nch) * 8],
                        nch * P, nch * P, felem, elem_step=felem,
                        queue_num=(t % NQ))
                for cc in range(nch):
                    c = c0 + cc
                    b, cloc = c // CPB, c % CPB
                    if cloc == 0:
                        start_block(b)
                    S, ph = active[b]
                    for h in range(nhalves):
                        nc.tensor.matmul(
                            out=ph[h][:],
                            lhsT=gt[:, cc, h * P:(h + 1) * P],
                            rhs=strided_cols(S[:], cloc, CPB, P),
                            start=(cloc == 0), stop=(cloc == CPB - 1))
                    if cloc == CPB - 1:
                        out_cb(b, ph)
                        del active[b]

        def l1_out(b, ph):
            hh = []
            for h in range(2):
                ht = hpool.tile([P, P], dt.bfloat16, tag=f"ht{h}")
                nc.scalar.activation(
                    ht[:], ph[h][:], mybir.ActivationFunctionType.Relu,
                    bias=b1_sb[:, h:h + 1])
                hh.append(ht)
            py = psum_y.tile([P, F_OUT], dt.float32, space="PSUM", tag="py")
            nc.tensor.matmul(out=py[:], lhsT=hh[0][:], rhs=w2_sb[:, 0:F_OUT],
                             start=True, stop=False)
            nc.tensor.matmul(out=py[:], lhsT=hh[1][:],
                             rhs=w2_sb[:, F_OUT:2 * F_OUT],
                             start=False, stop=True)
            yb = hpool.tile([P, F_OUT], dt.bfloat16, tag="yb")
            nc.vector.tensor_copy(yb[:], py[:])
            nc.sync.dma_start(yin[b * P:(b + 1) * P, :], yb[:])

        layer(zall, F_IN, 2, l1_out)

        if cfg.ncores > 1 and not os.environ.get("SKIP_COLL"):
            nc.gpsimd.collective_compute(
                "AllGather", mybir.AluOpType.bypass, replica_groups=rg,
                ins=[yin.opt()], outs=[yall.opt()])
        else:
            nc.sync.dma_start(yall[0:cfg.bp, :], yin[:])

        # ------- layer 2 -------
        def l2_out(b, ph):
            ob = hpool.tile([P, P], dt.float32, tag="ob")
            nc.scalar.activation(
                ob[:], ph[0][:], mybir.ActivationFunctionType.Identity,
                bias=b2_sb[:, 0:1])
            nc.sync.dma_start(outT[b], ob[:])

        layer(yall, F_OUT, 1, l2_out)


# --------------------------------------------------------------------------
# Top level
# --------------------------------------------------------------------------

def declare_io(nc, cfg):
    dt = mybir.dt
    def di(name, shape, d):
        return nc.dram_tensor(name, shape, d, kind="ExternalInput").ap()
    ins = {
        "featT": di("featT", [2, P, cfg.bp], dt.bfloat16),
        "w1": di("w1", [2, P, F_HID], dt.bfloat16),
        "w2": di("w2", [2, P, F_OUT], dt.bfloat16),
        "b1": di("b1", [2, P, 1], dt.float32),
        "b2": di("b2", [P, 1], dt.float32),
        "iota": di("iota", [P, cfg.cpb * P], dt.bfloat16),
        "idxs": di("idxs", [P, cfg.totch * 8], dt.int16),
        "segid": di("segid", [P, cfg.totch], dt.bfloat16),
        "wgt": di("wgt", [P, cfg.totch], dt.bfloat16),
    }
    outs = {
        "outT": nc.dram_tensor("outT", [cfg.blocks, F_OUT, P], dt.float32,
                               kind="ExternalOutput").ap(),
    }
    return ins, outs


def build_nc(cfg, meta, repeat=1):
    nc = bacc.Bacc("TRN2", target_bir_lowering=False, debug=False,
                   num_devices=cfg.ncores, num_swdge_queues=NQ)
    ins, outs = declare_io(nc, cfg)
    with tile.TileContext(nc) as tc:
        for _ in range(repeat):
            build_program(tc, cfg, meta, outs, ins)
    nc.compile()
    return nc


# --------------------------------------------------------------------------
# Harness entry point: kernel(**inputs) with FULL unsharded inputs
# --------------------------------------------------------------------------

def kernel(feat, row, col, edge_weight, W1, b1, W2, b2):
    feat = np.asarray(feat, dtype=np.float32)
    row = np.asarray(row, dtype=np.int32)
    col = np.asarray(col, dtype=np.int32)
    edge_weight = np.asarray(edge_weight, dtype=np.float32)
    W1 = np.asarray(W1, dtype=np.float32)
    b1 = np.asarray(b1, dtype=np.float32)
    W2 = np.asarray(W2, dtype=np.float32)
    b2 = np.asarray(b2, dtype=np.float32)

    cfg = full_cfg()
    assert feat.shape == (cfg.n, F_IN) and row.shape == (cfg.e,)

    in_maps, meta = preprocess(cfg, feat, row, col, edge_weight, W1, b1, W2, b2)
    nc = build_nc(cfg, meta)

    from concourse.bass_utils import run_bass_kernel_spmd
    res = run_bass_kernel_spmd(nc, in_maps, core_ids=list(range(cfg.ncores)))
    outs = [{"outT": r["outT"]} for r in res.results]
    return assemble(cfg, meta, outs)



# revision 28
# speedup vs baseline: 3.7457x; 1.0135x over previous
"""2-layer GCN (SpMM message passing) on 8 Trainium2 NeuronCores.

Strategy (row-sharded, gather-based):
  - Nodes are relabeled and assigned to (core, block, slot): 8 cores x BLOCKS
    blocks x 128 slots.  Assignment balances per-core edge counts and packs
    rows into blocks so every block's in-edge count fits CPB*128.
  - Each core computes Z = feat @ W1 for its own nodes (cast to fp8e4m3 to
    halve gather bytes; rel err ~0.006 vs the 0.02 gate), AllGathers the
    full Z into every core's HBM, then processes its own output rows:
    for each block, gather source rows Z[col] via dma_gather (256B rows),
    build an S matrix (one-hot(slot) * edge_weight) on DVE via two
    block-level tensor_tensors (2x_1p single-port mode -- per-chunk
    tensor_scalar would enter a 2-port DVE mode and starve SWDGE descriptor
    generation), and accumulate H_block^T = sum_chunks msgs^T @ S on the
    TensorEngine in PSUM.
  - H^T feeds Y = H @ W2 directly (lhsT = H^T), Y is AllGathered, layer 2
    repeats the same gather/aggregate with the same edge data.
  - dma_gather has int16 indices, so each gather call addresses a <=32767-row
    window of the Z/Y buffer; windows are static per call (shared across
    cores), chosen from the data (edges sorted by source within a block).
"""

import os
import sys
import numpy as np

sys.path.insert(0, "/opt/trn_rl_repo")

from concourse import bass, bacc, mybir, tile  # noqa: E402

P = 128
F_IN = 256
F_HID = 256
F_OUT = 128
WIN = 32767  # dma_gather int16 index window
MAX_GCALL = int(os.environ.get("MAX_GCALL", "8"))  # chunks per gather call
NQ = int(os.environ.get("NQ", "4"))  # SWDGE queues for gathers


def np_dt(dt):
    return mybir.dt.np(dt)


BF16 = np_dt(mybir.dt.bfloat16)


class Cfg:
    def __init__(self, n_nodes, n_edges, ncores, cpb, max_gcall=16):
        assert n_nodes % ncores == 0
        self.n = n_nodes
        self.e = n_edges
        self.ncores = ncores
        self.npc = n_nodes // ncores  # real nodes per core
        self.blocks = (self.npc + P - 1) // P
        if self.blocks * P - self.npc < 44:  # slack rows for bin packing
            self.blocks += 1
        self.cpb = cpb  # chunks (of 128 edges) per block
        self.max_gcall = max_gcall  # max chunks per dma_gather call
        self.totch = self.blocks * cpb  # total chunks per core
        self.bp = self.blocks * P  # padded nodes per core
        self.ntot = self.ncores * self.bp  # padded global nodes


def full_cfg():
    # capacity per core: blocks*cpb*128 must exceed max core edge count
    # 98 blocks * 32 chunks * 128 = 401408 > 400k + 2-3 sigma (assert-checked)
    return Cfg(100000, 3200000, 8, cpb=32, max_gcall=MAX_GCALL)


# --------------------------------------------------------------------------
# Host-side preprocessing
# --------------------------------------------------------------------------

def preprocess(cfg, feat, row, col, edge_weight, W1, b1, W2, b2):
    n, e = cfg.n, cfg.e
    deg_in = np.bincount(row, minlength=n)  # in-degree: edges aggregated per row

    # ---- assign nodes to cores (snake over degree-sorted nodes) ----
    order = np.argsort(-deg_in, kind="stable")
    ncores = cfg.ncores
    pos = np.arange(n)
    phase = pos % (2 * ncores)
    core_of_pos = np.where(phase < ncores, phase, 2 * ncores - 1 - phase)
    node_core = np.empty(n, dtype=np.int64)
    node_core[order] = core_of_pos

    # ---- within each core: snake nodes into blocks by degree ----
    node_block = np.empty(n, dtype=np.int64)
    node_slot = np.empty(n, dtype=np.int64)
    nb = cfg.blocks
    cap_edges = cfg.cpb * P
    for c in range(ncores):
        nodes = order[core_of_pos == c]  # degree-desc within core
        m = len(nodes)
        assert m == cfg.npc
        bpos = np.arange(m)
        ph = bpos % (2 * nb)
        blk = np.where(ph < nb, ph, 2 * nb - 1 - ph)
        # check block caps; fix overflows greedily
        cnt = np.bincount(blk, minlength=nb)
        esum = np.bincount(blk, weights=deg_in[nodes], minlength=nb)
        assert cnt.max() <= P, f"block row overflow {cnt.max()}"
        if esum.max() > cap_edges:
            blk = blk.copy()
            # move smallest-degree nodes out of overloaded blocks
            for b in np.where(esum > cap_edges)[0]:
                members = np.where(blk == b)[0]
                members = members[np.argsort(deg_in[nodes[members]])]
                k = 0
                while esum[b] > cap_edges and k < len(members):
                    mv = members[k]
                    d = deg_in[nodes[mv]]
                    cands = np.where(
                        (esum + d <= cap_edges) & (cnt < P))[0]
                    if len(cands) == 0:
                        raise RuntimeError("bin packing failed; raise cpb")
                    tgt = cands[np.argmin(esum[cands])]
                    blk[mv] = tgt
                    esum[b] -= d
                    esum[tgt] += d
                    cnt[b] -= 1
                    cnt[tgt] += 1
                    k += 1
            assert esum.max() <= cap_edges
        # slots within block
        slot = np.zeros(m, dtype=np.int64)
        so = np.argsort(blk, kind="stable")
        sb = blk[so]
        start = np.r_[0, np.flatnonzero(np.diff(sb)) + 1]
        sizes = np.diff(np.r_[start, m])
        ranks = np.arange(m) - np.repeat(start, sizes)
        slot[so] = ranks
        node_block[nodes] = blk
        node_slot[nodes] = slot

    newid = node_core * cfg.bp + node_block * P + node_slot

    # ---- edges: assign to (core, block) of their destination row ----
    er_new = newid[row]
    ec_new = newid[col]
    e_core = node_core[row]
    e_blk = node_block[row]
    e_slot = node_slot[row]
    gblk = e_core * cfg.blocks + e_blk  # global block id
    # serpentine: odd blocks sorted by descending col so gather calls can
    # span block boundaries within one int16 window
    sortcol = np.where(e_blk % 2 == 0, ec_new, cfg.ntot - 1 - ec_new)
    so = np.lexsort((sortcol, gblk))
    gblk_s = gblk[so]
    ec_s = ec_new[so]
    slot_s = e_slot[so]
    w_s = edge_weight[so]

    # per-core padded edge stream
    tot = cfg.totch * P
    nblk_g = ncores * cfg.blocks
    blk_cnt = np.bincount(gblk_s, minlength=nblk_g)
    assert blk_cnt.max() <= cap_edges, f"block edges {blk_cnt.max()} > {cap_edges}"
    blk_start = np.r_[0, np.cumsum(blk_cnt)[:-1]]
    rank_in_blk = np.arange(len(so)) - np.repeat(blk_start, blk_cnt)
    stream_pos = (gblk_s % cfg.blocks) * cap_edges + rank_in_blk  # within-core pos
    core_of_edge = gblk_s // cfg.blocks

    colpad = np.zeros((ncores, tot), dtype=np.int64)
    slotpad = np.zeros((ncores, tot), dtype=np.int64)
    wpad = np.zeros((ncores, tot), dtype=np.float32)
    validpad = np.zeros((ncores, tot), dtype=bool)
    colpad[core_of_edge, stream_pos] = ec_s
    slotpad[core_of_edge, stream_pos] = slot_s
    wpad[core_of_edge, stream_pos] = w_s
    validpad[core_of_edge, stream_pos] = True

    # ---- greedy gather-call plan (shared across cores) ----
    # Walk the chunk stream; pack consecutive chunks into one dma_gather call
    # while the cross-core col span of real edges stays within the int16
    # window and the call has at most max_gcall chunks.
    big = np.int64(1 << 60)
    nvc = validpad.reshape(ncores, cfg.totch, P)
    cvc = colpad.reshape(ncores, cfg.totch, P)
    ch_min = np.where(nvc, cvc, big).min(axis=(0, 2))  # per-chunk min col
    ch_max = np.where(nvc, cvc, -1).max(axis=(0, 2))
    call_start = []  # first chunk of each call
    call_len = []
    cur_lo, cur_hi = big, -1
    cur0, curn = 0, 0
    for t in range(cfg.totch):
        lo = min(cur_lo, ch_min[t])
        hi = max(cur_hi, ch_max[t])
        span_ok = (hi < 0) or (lo == big) or (hi - lo <= WIN - 1)
        if curn > 0 and (curn >= cfg.max_gcall or not span_ok):
            call_start.append(cur0)
            call_len.append(curn)
            cur0, curn = t, 0
            cur_lo, cur_hi = big, -1
            lo, hi = ch_min[t], ch_max[t]
        cur_lo, cur_hi = lo, hi
        curn += 1
    call_start.append(cur0)
    call_len.append(curn)
    calls = len(call_start)
    call_start = np.array(call_start)
    call_len = np.array(call_len)

    base = np.zeros(calls, dtype=np.int64)
    width = np.zeros(calls, dtype=np.int64)
    chunk_base = np.zeros(cfg.totch, dtype=np.int64)
    for t in range(calls):
        sl = slice(call_start[t], call_start[t] + call_len[t])
        lo = np.where(nvc[:, sl], cvc[:, sl], big).min()
        lo = 0 if lo == big else lo
        base[t] = min(lo, max(cfg.ntot - WIN, 0))
        width[t] = min(WIN, cfg.ntot - base[t])
        hi = np.where(nvc[:, sl], cvc[:, sl], -1).max()
        assert hi < base[t] + width[t], f"window overflow call {t}"
        chunk_base[sl] = base[t]

    # pad entries: idx = base of their call (always in-window), w = 0
    colpad = np.where(validpad, colpad,
                      np.broadcast_to(np.repeat(chunk_base, P), (ncores, tot)))
    wpad = np.where(validpad, wpad, 0.0)

    # ---- per-core device input planes ----
    idx16 = (colpad.reshape(ncores, cfg.totch, P)
             - np.repeat(chunk_base, P).reshape(cfg.totch, P)[None]).astype(np.int16)
    assert (idx16 >= 0).all()
    # idx plane: [128, totch*8]; per chunk t an [128, 8] block with
    # tile[p, t*8 + j] = idx[t, j*16 + p%16] -- calls read contiguous spans
    idxp = idx16.reshape(ncores, cfg.totch, 8, 16)
    idxp = idxp.transpose(0, 3, 1, 2).reshape(ncores, 16, cfg.totch * 8)
    idx_plane = np.tile(idxp, (1, 8, 1))  # replicate to 128 partitions

    # segid/w planes: [128, totch]; [p, t] = edge (t*128+p); bf16 (values 0..127
    # are exact) so the S-generation tensor_tensor qualifies for DVE 2x mode
    seg_plane = slotpad.reshape(ncores, cfg.totch, P).transpose(0, 2, 1)
    seg_plane = np.ascontiguousarray(seg_plane).astype(BF16)
    w_plane = wpad.reshape(ncores, cfg.totch, P).transpose(0, 2, 1)
    w_plane = np.ascontiguousarray(w_plane).astype(BF16)

    # iotaT plane [128, 128*cpb] bf16: [p, s*cpb + c] = s  (transposed S layout)
    iota_plane = np.repeat(np.arange(P), cfg.cpb).astype(BF16)
    iota_plane = np.tile(iota_plane, (P, 1))

    # featT planes [2, 128, bp] bf16 per core
    feat_pad = np.zeros((cfg.ntot, F_IN), dtype=np.float32)
    feat_pad[newid] = feat
    featT = np.ascontiguousarray(
        feat_pad.reshape(ncores, cfg.bp, 2, P).transpose(0, 2, 3, 1)
    ).astype(BF16)  # [ncores, 2, 128, bp]

    w1p = np.ascontiguousarray(W1.reshape(2, P, F_HID)).astype(BF16)
    w2p = np.ascontiguousarray(W2.reshape(2, P, F_OUT)).astype(BF16)
    b1p = np.ascontiguousarray(b1.reshape(2, P, 1)).astype(np.float32)
    b2p = np.ascontiguousarray(b2.reshape(P, 1)).astype(np.float32)

    in_maps = []
    for c in range(ncores):
        in_maps.append({
            "featT": featT[c],
            "w1": w1p, "w2": w2p, "b1": b1p, "b2": b2p,
            "iota": iota_plane,
            "idxs": np.ascontiguousarray(idx_plane[c]),
            "segid": seg_plane[c],
            "wgt": w_plane[c],
        })
    meta = {
        "base": base.astype(np.int64),
        "width": width.astype(np.int64),
        "call_start": call_start,
        "call_len": call_len,
        "newid": newid,
        "node_core": node_core,
        "node_block": node_block,
        "node_slot": node_slot,
    }
    return in_maps, meta


def assemble(cfg, meta, outs):
    """outs: list per core of {'outT': [blocks,128,128] f32} -> [n, F_OUT]."""
    res = np.empty((cfg.n, F_OUT), dtype=np.float32)
    nc_, nb_, ns_ = meta["node_core"], meta["node_block"], meta["node_slot"]
    for c in range(cfg.ncores):
        o = outs[c]["outT"]  # [blocks, F_OUT, 128]
        sel = np.where(nc_ == c)[0]
        res[sel] = o[nb_[sel], :, ns_[sel]]
    return res


# --------------------------------------------------------------------------
# Device program
# --------------------------------------------------------------------------

def inner_bcast(ap, k):
    """Append a step-0 dim of size k to an AP (broadcast each element k times)."""
    return bass.AP(ap.tensor, ap.offset, list(ap.ap) + [[0, k]])


def outer_bcast(ap, k):
    """Insert a step-0 dim of size k before the last dim (repeat the last-dim
    sequence k times). Keeps the last dim contiguous for DVE 2x mode."""
    a = list(ap.ap)
    return bass.AP(ap.tensor, ap.offset, a[:-1] + [[0, k]] + a[-1:])


def strided_cols(ap, start, step, count):
    """Column view [p, start + step*j] of a contiguous [128, N] AP."""
    a = list(ap.ap)
    return bass.AP(ap.tensor, ap.offset + start, [a[0], [step, count]])


def build_program(tc, cfg, meta, outs, ins):
    nc = tc.nc
    dt = mybir.dt
    base, width = meta["base"], meta["width"]
    call_start, call_len = meta["call_start"], meta["call_len"]
    featT, w1, w2 = ins["featT"], ins["w1"], ins["w2"]
    b1, b2 = ins["b1"], ins["b2"]
    iota, idxs, segid, wgt = ins["iota"], ins["idxs"], ins["segid"], ins["wgt"]
    outT = outs["outT"]
    BL, CPB = cfg.blocks, cfg.cpb
    rg = [list(range(cfg.ncores))]

    with tc.tile_pool(name="const", bufs=1) as const, \
         tc.tile_pool(name="dram", bufs=1, space="DRAM") as dram, \
         tc.tile_pool(name="gpool", bufs=6) as gpool, \
         tc.tile_pool(name="spool", bufs=2) as spool, \
         tc.tile_pool(name="hpool", bufs=3) as hpool, \
         tc.tile_pool(name="psum_h", bufs=2, space="PSUM") as psum_h, \
         tc.tile_pool(name="psum_y", bufs=2, space="PSUM") as psum_y:
        # ------- constants -------
        idx_sb = const.tile([P, cfg.totch * 8], dt.int16)
        nc.sync.dma_start(idx_sb[:], idxs[:])
        seg_sb = const.tile([P, cfg.totch], dt.bfloat16)
        nc.sync.dma_start(seg_sb[:], segid[:])
        w_sb = const.tile([P, cfg.totch], dt.bfloat16)
        nc.sync.dma_start(w_sb[:], wgt[:])
        iota_sb = const.tile([P, CPB * P], dt.bfloat16)
        nc.sync.dma_start(iota_sb[:], iota[:])
        w1_sb = const.tile([P, 2 * F_HID], dt.bfloat16)
        nc.sync.dma_start(w1_sb[:, 0:F_HID], w1[0])
        nc.sync.dma_start(w1_sb[:, F_HID:2 * F_HID], w1[1])
        w2_sb = const.tile([P, 2 * F_OUT], dt.bfloat16)
        nc.sync.dma_start(w2_sb[:, 0:F_OUT], w2[0])
        nc.sync.dma_start(w2_sb[:, F_OUT:2 * F_OUT], w2[1])
        b1_sb = const.tile([P, 2], dt.float32)
        nc.sync.dma_start(b1_sb[:, 0:1], b1[0])
        nc.sync.dma_start(b1_sb[:, 1:2], b1[1])
        b2_sb = const.tile([P, 1], dt.float32)
        nc.sync.dma_start(b2_sb[:], b2[:])

        zdt = dt.bfloat16 if os.environ.get("NO_FP8") else dt.float8e4
        zin = dram.tile([cfg.bp, F_IN], zdt)
        zall = dram.tile([cfg.ntot, F_IN], zdt, addr_space="Shared")
        yin = dram.tile([cfg.bp, F_OUT], dt.bfloat16)
        yall = dram.tile([cfg.ntot, F_OUT], dt.bfloat16, addr_space="Shared")

        # ------- phase Z: Z = feat @ W1 for own shard -------
        with tc.tile_pool(name="zpool", bufs=3) as zpool, \
             tc.tile_pool(name="ftpool", bufs=1) as ftpool, \
             tc.tile_pool(name="psum_z", bufs=2, space="PSUM") as psum_z:
            ft_sb = ftpool.tile([P, 2 * cfg.bp], dt.bfloat16)
            nc.sync.dma_start(ft_sb[:, 0:cfg.bp], featT[0])
            nc.sync.dma_start(ft_sb[:, cfg.bp:2 * cfg.bp], featT[1])
            for g in range(BL):
                pz = psum_z.tile([P, F_HID], dt.float32, space="PSUM", tag="pz")
                nc.tensor.matmul(
                    out=pz[:], lhsT=ft_sb[:, g * P:(g + 1) * P],
                    rhs=w1_sb[:, 0:F_HID], start=True, stop=False)
                nc.tensor.matmul(
                    out=pz[:], lhsT=ft_sb[:, cfg.bp + g * P:cfg.bp + (g + 1) * P],
                    rhs=w1_sb[:, F_HID:2 * F_HID], start=False, stop=True)
                zb = zpool.tile([P, F_HID], zdt, tag="zb")
                nc.vector.tensor_copy(zb[:], pz[:])
                nc.sync.dma_start(zin[g * P:(g + 1) * P, :], zb[:])

        if cfg.ncores > 1 and not os.environ.get("SKIP_COLL"):
            nc.gpsimd.collective_compute(
                "AllGather", mybir.AluOpType.bypass, replica_groups=rg,
                ins=[zin.opt()], outs=[zall.opt()])
        else:
            nc.sync.dma_start(zall[0:cfg.bp, :], zin[:])

        # ------- layer 1 + Y -------
        def layer(src, felem, gdt, nhalves, out_cb):
            active = {}  # block -> (S, ph list)

            def start_block(b):
                # S in transposed layout: S[p, s*CPB + c] = w[p,c]*(slot==s).
                # All operands keep a contiguous last dim -> DVE 2x mode.
                eq = spool.tile([P, CPB * P], dt.bfloat16, tag="eq", name="eq")
                S = spool.tile([P, CPB * P], dt.bfloat16, tag="S", name="S")
                if os.environ.get("SKIP_SGEN"):
                    nc.vector.tensor_copy(S[:], iota_sb[:, 0:CPB * P])
                    active[b] = (S, [psum_h.tile(
                        [P, P], dt.float32, space="PSUM", tag=f"ph{h}",
                        name=f"ph{h}") for h in range(nhalves)])
                    return
                nc.vector.tensor_tensor(
                    out=eq[:], in0=iota_sb[:],
                    in1=outer_bcast(seg_sb[:, b * CPB:(b + 1) * CPB], P),
                    op=mybir.AluOpType.is_equal)
                nc.vector.tensor_tensor(
                    out=S[:], in0=eq[:],
                    in1=outer_bcast(w_sb[:, b * CPB:(b + 1) * CPB], P),
                    op=mybir.AluOpType.mult)
                ph = [psum_h.tile([P, P], dt.float32, space="PSUM",
                                  tag=f"ph{h}", name=f"ph{h}")
                      for h in range(nhalves)]
                active[b] = (S, ph)

            for t in range(len(call_start)):
                c0, nch = int(call_start[t]), int(call_len[t])
                gt = gpool.tile([P, cfg.max_gcall, felem], gdt,
                                tag="gt", name="gt")
                win = src[int(base[t]):int(base[t] + width[t]), :]
                if os.environ.get("GATHER_HALF") and felem == 256:
                    # same calls/descriptors, half the bytes (junk numerics)
                    gh = gpool.tile([P, cfg.max_gcall, felem // 2],
                                    dt.bfloat16, tag="gh", name="gh")
                    nc.gpsimd.dma_gather(
                        gh[:, :nch, :], win, idx_sb[:, c0 * 8:(c0 + nch) * 8],
                        nch * P, nch * P, felem // 2, elem_step=felem,
                        queue_num=(t % NQ))
                    nc.vector.tensor_copy(gt[:, 0, 0:P], iota_sb[:, 0:P])
                else:
                    nc.gpsimd.dma_gather(
                        gt[:, :nch, :], win, idx_sb[:, c0 * 8:(c0 + nch) * 8],
                        nch * P, nch * P, felem, elem_step=felem,
                        queue_num=(t % NQ))
                for cc in range(nch):
                    c = c0 + cc
                    b, cloc = c // CPB, c % CPB
                    if cloc == 0:
                        start_block(b)
                    S, ph = active[b]
                    for h in range(nhalves):
                        nc.tensor.matmul(
                            out=ph[h][:],
                            lhsT=gt[:, cc, h * P:(h + 1) * P],
                            rhs=strided_cols(S[:], cloc, CPB, P),
                            start=(cloc == 0), stop=(cloc == CPB - 1))
                    if cloc == CPB - 1:
                        out_cb(b, ph)
                        del active[b]

        def l1_out(b, ph):
            hh = []
            for h in range(2):
                ht = hpool.tile([P, P], dt.bfloat16, tag=f"ht{h}")
                nc.scalar.activation(
                    ht[:], ph[h][:], mybir.ActivationFunctionType.Relu,
                    bias=b1_sb[:, h:h + 1])
                hh.append(ht)
            py = psum_y.tile([P, F_OUT], dt.float32, space="PSUM", tag="py")
            nc.tensor.matmul(out=py[:], lhsT=hh[0][:], rhs=w2_sb[:, 0:F_OUT],
                             start=True, stop=False)
            nc.tensor.matmul(out=py[:], lhsT=hh[1][:],
                             rhs=w2_sb[:, F_OUT:2 * F_OUT],
                             start=False, stop=True)
            yb = hpool.tile([P, F_OUT], dt.bfloat16, tag="yb")
            nc.vector.tensor_copy(yb[:], py[:])
            nc.sync.dma_start(yin[b * P:(b + 1) * P, :], yb[:])

        layer(zall, F_IN, zdt, 2, l1_out)

        if cfg.ncores > 1 and not os.environ.get("SKIP_COLL"):
            nc.gpsimd.collective_compute(
                "AllGather", mybir.AluOpType.bypass, replica_groups=rg,
                ins=[yin.opt()], outs=[yall.opt()])
        else:
            nc.sync.dma_start(yall[0:cfg.bp, :], yin[:])

        # ------- layer 2 -------
        def l2_out(b, ph):
            ob = hpool.tile([P, P], dt.float32, tag="ob")
            nc.scalar.activation(
                ob[:], ph[0][:], mybir.ActivationFunctionType.Identity,
                bias=b2_sb[:, 0:1])
            nc.sync.dma_start(outT[b], ob[:])

        layer(yall, F_OUT, dt.bfloat16, 1, l2_out)


# --------------------------------------------------------------------------
# Top level
# --------------------------------------------------------------------------

def declare_io(nc, cfg):
    dt = mybir.dt
    def di(name, shape, d):
        return nc.dram_tensor(name, shape, d, kind="ExternalInput").ap()
    ins = {
        "featT": di("featT", [2, P, cfg.bp], dt.bfloat16),
        "w1": di("w1", [2, P, F_HID], dt.bfloat16),
        "w2": di("w2", [2, P, F_OUT], dt.bfloat16),
        "b1": di("b1", [2, P, 1], dt.float32),
        "b2": di("b2", [P, 1], dt.float32),
        "iota": di("iota", [P, cfg.cpb * P], dt.bfloat16),
        "idxs": di("idxs", [P, cfg.totch * 8], dt.int16),
        "segid": di("segid", [P, cfg.totch], dt.bfloat16),
        "wgt": di("wgt", [P, cfg.totch], dt.bfloat16),
    }
    outs = {
        "outT": nc.dram_tensor("outT", [cfg.blocks, F_OUT, P], dt.float32,
                               kind="ExternalOutput").ap(),
    }
    return ins, outs


def build_nc(cfg, meta, repeat=1):
    nc = bacc.Bacc("TRN2", target_bir_lowering=False, debug=False,
                   num_devices=cfg.ncores, num_swdge_queues=NQ)
    ins, outs = declare_io(nc, cfg)
    with tile.TileContext(nc) as tc:
        for _ in range(repeat):
            build_program(tc, cfg, meta, outs, ins)
    nc.compile()
    return nc


# --------------------------------------------------------------------------
# Harness entry point: kernel(**inputs) with FULL unsharded inputs
# --------------------------------------------------------------------------

def kernel(feat, row, col, edge_weight, W1, b1, W2, b2):
    feat = np.asarray(feat, dtype=np.float32)
    row = np.asarray(row, dtype=np.int32)
    col = np.asarray(col, dtype=np.int32)
    edge_weight = np.asarray(edge_weight, dtype=np.float32)
    W1 = np.asarray(W1, dtype=np.float32)
    b1 = np.asarray(b1, dtype=np.float32)
    W2 = np.asarray(W2, dtype=np.float32)
    b2 = np.asarray(b2, dtype=np.float32)

    cfg = full_cfg()
    assert feat.shape == (cfg.n, F_IN) and row.shape == (cfg.e,)

    in_maps, meta = preprocess(cfg, feat, row, col, edge_weight, W1, b1, W2, b2)
    nc = build_nc(cfg, meta)

    from concourse.bass_utils import run_bass_kernel_spmd
    res = run_bass_kernel_spmd(nc, in_maps, core_ids=list(range(cfg.ncores)))
    outs = [{"outT": r["outT"]} for r in res.results]
    return assemble(cfg, meta, outs)



# revision 30
# speedup vs baseline: 3.9303x; 1.0493x over previous
"""2-layer GCN (SpMM message passing) on 8 Trainium2 NeuronCores.

Strategy (row-sharded, gather-based):
  - Nodes are relabeled and assigned to (core, block, slot): 8 cores x BLOCKS
    blocks x 128 slots.  Assignment balances per-core edge counts and packs
    rows into blocks so every block's in-edge count fits CPB*128.
  - Each core computes Z = feat @ W1 for its own nodes (cast to fp8e4m3 to
    halve gather bytes; rel err ~0.006 vs the 0.02 gate), AllGathers the
    full Z into every core's HBM, then processes its own output rows:
    for each block, gather source rows Z[col] via dma_gather (256B rows),
    build an S matrix (one-hot(slot) * edge_weight) on DVE via two
    block-level tensor_tensors (2x_1p single-port mode -- per-chunk
    tensor_scalar would enter a 2-port DVE mode and starve SWDGE descriptor
    generation), and accumulate H_block^T = sum_chunks msgs^T @ S on the
    TensorEngine in PSUM.
  - H^T feeds Y = H @ W2 directly (lhsT = H^T), Y is AllGathered, layer 2
    repeats the same gather/aggregate with the same edge data.
  - dma_gather has int16 indices, so each gather call addresses a <=32767-row
    window of the Z/Y buffer; windows are static per call (shared across
    cores), chosen from the data (edges sorted by source within a block).
"""

import os
import sys
import numpy as np

sys.path.insert(0, "/opt/trn_rl_repo")

from concourse import bass, bacc, mybir, tile  # noqa: E402

P = 128
F_IN = 256
F_HID = 256
F_OUT = 128
WIN = 32767  # dma_gather int16 index window
MAX_GCALL = int(os.environ.get("MAX_GCALL", "8"))  # chunks per gather call
NQ = int(os.environ.get("NQ", "4"))  # SWDGE queues for gathers


def np_dt(dt):
    return mybir.dt.np(dt)


BF16 = np_dt(mybir.dt.bfloat16)


class Cfg:
    def __init__(self, n_nodes, n_edges, ncores, cpb, max_gcall=16):
        assert n_nodes % ncores == 0
        self.n = n_nodes
        self.e = n_edges
        self.ncores = ncores
        self.npc = n_nodes // ncores  # real nodes per core
        self.blocks = (self.npc + P - 1) // P
        if self.blocks * P - self.npc < 44:  # slack rows for bin packing
            self.blocks += 1
        self.cpb = cpb  # chunks (of 128 edges) per block
        self.max_gcall = max_gcall  # max chunks per dma_gather call
        self.totch = self.blocks * cpb  # total chunks per core
        self.bp = self.blocks * P  # padded nodes per core
        self.ntot = self.ncores * self.bp  # padded global nodes


def full_cfg():
    # capacity per core: blocks*cpb*128 must exceed max core edge count
    # 98 blocks * 32 chunks * 128 = 401408 > 400k + 2-3 sigma (assert-checked)
    return Cfg(100000, 3200000, 8, cpb=32, max_gcall=MAX_GCALL)


# --------------------------------------------------------------------------
# Host-side preprocessing
# --------------------------------------------------------------------------

def preprocess(cfg, feat, row, col, edge_weight, W1, b1, W2, b2):
    n, e = cfg.n, cfg.e
    deg_in = np.bincount(row, minlength=n)  # in-degree: edges aggregated per row

    # ---- assign nodes to cores (snake over degree-sorted nodes) ----
    order = np.argsort(-deg_in, kind="stable")
    ncores = cfg.ncores
    pos = np.arange(n)
    phase = pos % (2 * ncores)
    core_of_pos = np.where(phase < ncores, phase, 2 * ncores - 1 - phase)
    node_core = np.empty(n, dtype=np.int64)
    node_core[order] = core_of_pos

    # ---- within each core: snake nodes into blocks by degree ----
    node_block = np.empty(n, dtype=np.int64)
    node_slot = np.empty(n, dtype=np.int64)
    nb = cfg.blocks
    cap_edges = cfg.cpb * P
    for c in range(ncores):
        nodes = order[core_of_pos == c]  # degree-desc within core
        m = len(nodes)
        assert m == cfg.npc
        bpos = np.arange(m)
        ph = bpos % (2 * nb)
        blk = np.where(ph < nb, ph, 2 * nb - 1 - ph)
        # check block caps; fix overflows greedily
        cnt = np.bincount(blk, minlength=nb)
        esum = np.bincount(blk, weights=deg_in[nodes], minlength=nb)
        assert cnt.max() <= P, f"block row overflow {cnt.max()}"
        if esum.max() > cap_edges:
            blk = blk.copy()
            # move smallest-degree nodes out of overloaded blocks
            for b in np.where(esum > cap_edges)[0]:
                members = np.where(blk == b)[0]
                members = members[np.argsort(deg_in[nodes[members]])]
                k = 0
                while esum[b] > cap_edges and k < len(members):
                    mv = members[k]
                    d = deg_in[nodes[mv]]
                    cands = np.where(
                        (esum + d <= cap_edges) & (cnt < P))[0]
                    if len(cands) == 0:
                        raise RuntimeError("bin packing failed; raise cpb")
                    tgt = cands[np.argmin(esum[cands])]
                    blk[mv] = tgt
                    esum[b] -= d
                    esum[tgt] += d
                    cnt[b] -= 1
                    cnt[tgt] += 1
                    k += 1
            assert esum.max() <= cap_edges
        # slots within block
        slot = np.zeros(m, dtype=np.int64)
        so = np.argsort(blk, kind="stable")
        sb = blk[so]
        start = np.r_[0, np.flatnonzero(np.diff(sb)) + 1]
        sizes = np.diff(np.r_[start, m])
        ranks = np.arange(m) - np.repeat(start, sizes)
        slot[so] = ranks
        node_block[nodes] = blk
        node_slot[nodes] = slot

    newid = node_core * cfg.bp + node_block * P + node_slot

    # ---- edges: assign to (core, block) of their destination row ----
    er_new = newid[row]
    ec_new = newid[col]
    e_core = node_core[row]
    e_blk = node_block[row]
    e_slot = node_slot[row]
    gblk = e_core * cfg.blocks + e_blk  # global block id
    # serpentine: odd blocks sorted by descending col so gather calls can
    # span block boundaries within one int16 window
    sortcol = np.where(e_blk % 2 == 0, ec_new, cfg.ntot - 1 - ec_new)
    so = np.lexsort((sortcol, gblk))
    gblk_s = gblk[so]
    ec_s = ec_new[so]
    slot_s = e_slot[so]
    w_s = edge_weight[so]

    # per-core padded edge stream
    tot = cfg.totch * P
    nblk_g = ncores * cfg.blocks
    blk_cnt = np.bincount(gblk_s, minlength=nblk_g)
    assert blk_cnt.max() <= cap_edges, f"block edges {blk_cnt.max()} > {cap_edges}"
    blk_start = np.r_[0, np.cumsum(blk_cnt)[:-1]]
    rank_in_blk = np.arange(len(so)) - np.repeat(blk_start, blk_cnt)
    stream_pos = (gblk_s % cfg.blocks) * cap_edges + rank_in_blk  # within-core pos
    core_of_edge = gblk_s // cfg.blocks

    colpad = np.zeros((ncores, tot), dtype=np.int64)
    slotpad = np.zeros((ncores, tot), dtype=np.int64)
    wpad = np.zeros((ncores, tot), dtype=np.float32)
    validpad = np.zeros((ncores, tot), dtype=bool)
    colpad[core_of_edge, stream_pos] = ec_s
    slotpad[core_of_edge, stream_pos] = slot_s
    wpad[core_of_edge, stream_pos] = w_s
    validpad[core_of_edge, stream_pos] = True

    # ---- greedy gather-call plan (shared across cores) ----
    # Walk the chunk stream; pack consecutive chunks into one dma_gather call
    # while the cross-core col span of real edges stays within the int16
    # window and the call has at most max_gcall chunks.
    big = np.int64(1 << 60)
    nvc = validpad.reshape(ncores, cfg.totch, P)
    cvc = colpad.reshape(ncores, cfg.totch, P)
    ch_min = np.where(nvc, cvc, big).min(axis=(0, 2))  # per-chunk min col
    ch_max = np.where(nvc, cvc, -1).max(axis=(0, 2))
    call_start = []  # first chunk of each call
    call_len = []
    cur_lo, cur_hi = big, -1
    cur0, curn = 0, 0
    for t in range(cfg.totch):
        lo = min(cur_lo, ch_min[t])
        hi = max(cur_hi, ch_max[t])
        span_ok = (hi < 0) or (lo == big) or (hi - lo <= WIN - 1)
        if curn > 0 and (curn >= cfg.max_gcall or not span_ok):
            call_start.append(cur0)
            call_len.append(curn)
            cur0, curn = t, 0
            cur_lo, cur_hi = big, -1
            lo, hi = ch_min[t], ch_max[t]
        cur_lo, cur_hi = lo, hi
        curn += 1
    call_start.append(cur0)
    call_len.append(curn)
    calls = len(call_start)
    call_start = np.array(call_start)
    call_len = np.array(call_len)

    base = np.zeros(calls, dtype=np.int64)
    width = np.zeros(calls, dtype=np.int64)
    chunk_base = np.zeros(cfg.totch, dtype=np.int64)
    for t in range(calls):
        sl = slice(call_start[t], call_start[t] + call_len[t])
        lo = np.where(nvc[:, sl], cvc[:, sl], big).min()
        lo = 0 if lo == big else lo
        base[t] = min(lo, max(cfg.ntot - WIN, 0))
        width[t] = min(WIN, cfg.ntot - base[t])
        hi = np.where(nvc[:, sl], cvc[:, sl], -1).max()
        assert hi < base[t] + width[t], f"window overflow call {t}"
        chunk_base[sl] = base[t]

    # pad entries: idx = base of their call (always in-window), w = 0
    colpad = np.where(validpad, colpad,
                      np.broadcast_to(np.repeat(chunk_base, P), (ncores, tot)))
    wpad = np.where(validpad, wpad, 0.0)

    # ---- per-core device input planes ----
    idx16 = (colpad.reshape(ncores, cfg.totch, P)
             - np.repeat(chunk_base, P).reshape(cfg.totch, P)[None]).astype(np.int16)
    assert (idx16 >= 0).all()
    # idx plane: [128, totch*8]; per chunk t an [128, 8] block with
    # tile[p, t*8 + j] = idx[t, j*16 + p%16] -- calls read contiguous spans
    idxp = idx16.reshape(ncores, cfg.totch, 8, 16)
    idxp = idxp.transpose(0, 3, 1, 2).reshape(ncores, 16, cfg.totch * 8)
    idx_plane = np.tile(idxp, (1, 8, 1))  # replicate to 128 partitions

    # segid/w planes: [128, totch]; [p, t] = edge (t*128+p); bf16 (values 0..127
    # are exact) so the S-generation tensor_tensor qualifies for DVE 2x mode
    seg_plane = slotpad.reshape(ncores, cfg.totch, P).transpose(0, 2, 1)
    seg_plane = np.ascontiguousarray(seg_plane).astype(BF16)
    w_plane = wpad.reshape(ncores, cfg.totch, P).transpose(0, 2, 1)
    w_plane = np.ascontiguousarray(w_plane).astype(BF16)

    # iotaT plane [128, 128*cpb] bf16: [p, s*cpb + c] = s  (transposed S layout)
    iota_plane = np.repeat(np.arange(P), cfg.cpb).astype(BF16)
    iota_plane = np.tile(iota_plane, (P, 1))

    # featT planes [2, 128, bp] bf16 per core
    feat_pad = np.zeros((cfg.ntot, F_IN), dtype=np.float32)
    feat_pad[newid] = feat
    featT = np.ascontiguousarray(
        feat_pad.reshape(ncores, cfg.bp, 2, P).transpose(0, 2, 3, 1)
    ).astype(BF16)  # [ncores, 2, 128, bp]

    w1p = np.ascontiguousarray(W1.reshape(2, P, F_HID)).astype(BF16)
    w2p = np.ascontiguousarray(W2.reshape(2, P, F_OUT)).astype(BF16)
    b1p = np.ascontiguousarray(b1.reshape(2, P, 1)).astype(np.float32)
    b2p = np.ascontiguousarray(b2.reshape(P, 1)).astype(np.float32)

    in_maps = []
    for c in range(ncores):
        in_maps.append({
            "featT": featT[c],
            "w1": w1p, "w2": w2p, "b1": b1p, "b2": b2p,
            "iota": iota_plane,
            "idxs": np.ascontiguousarray(idx_plane[c]),
            "segid": seg_plane[c],
            "wgt": w_plane[c],
        })
    meta = {
        "base": base.astype(np.int64),
        "width": width.astype(np.int64),
        "call_start": call_start,
        "call_len": call_len,
        "newid": newid,
        "node_core": node_core,
        "node_block": node_block,
        "node_slot": node_slot,
    }
    return in_maps, meta


def assemble(cfg, meta, outs):
    """outs: list per core of {'outT': [blocks,128,128] f32} -> [n, F_OUT]."""
    res = np.empty((cfg.n, F_OUT), dtype=np.float32)
    nc_, nb_, ns_ = meta["node_core"], meta["node_block"], meta["node_slot"]
    for c in range(cfg.ncores):
        o = outs[c]["outT"]  # [blocks, F_OUT, 128]
        sel = np.where(nc_ == c)[0]
        res[sel] = o[nb_[sel], :, ns_[sel]]
    return res


# --------------------------------------------------------------------------
# Device program
# --------------------------------------------------------------------------

def inner_bcast(ap, k):
    """Append a step-0 dim of size k to an AP (broadcast each element k times)."""
    return bass.AP(ap.tensor, ap.offset, list(ap.ap) + [[0, k]])


def outer_bcast(ap, k):
    """Insert a step-0 dim of size k before the last dim (repeat the last-dim
    sequence k times). Keeps the last dim contiguous for DVE 2x mode."""
    a = list(ap.ap)
    return bass.AP(ap.tensor, ap.offset, a[:-1] + [[0, k]] + a[-1:])


def strided_cols(ap, start, step, count):
    """Column view [p, start + step*j] of a contiguous [128, N] AP."""
    a = list(ap.ap)
    return bass.AP(ap.tensor, ap.offset + start, [a[0], [step, count]])


def build_program(tc, cfg, meta, outs, ins):
    nc = tc.nc
    dt = mybir.dt
    base, width = meta["base"], meta["width"]
    call_start, call_len = meta["call_start"], meta["call_len"]
    featT, w1, w2 = ins["featT"], ins["w1"], ins["w2"]
    b1, b2 = ins["b1"], ins["b2"]
    iota, idxs, segid, wgt = ins["iota"], ins["idxs"], ins["segid"], ins["wgt"]
    outT = outs["outT"]
    BL, CPB = cfg.blocks, cfg.cpb
    rg = [list(range(cfg.ncores))]

    with tc.tile_pool(name="const", bufs=1) as const, \
         tc.tile_pool(name="dram", bufs=1, space="DRAM") as dram, \
         tc.tile_pool(name="gpool", bufs=8) as gpool, \
         tc.tile_pool(name="spool", bufs=3) as spool, \
         tc.tile_pool(name="hpool", bufs=3) as hpool, \
         tc.tile_pool(name="psum_h", bufs=2, space="PSUM") as psum_h, \
         tc.tile_pool(name="psum_y", bufs=2, space="PSUM") as psum_y:
        # ------- constants -------
        idx_sb = const.tile([P, cfg.totch * 8], dt.int16)
        nc.sync.dma_start(idx_sb[:], idxs[:])
        seg_sb = const.tile([P, cfg.totch], dt.bfloat16)
        nc.sync.dma_start(seg_sb[:], segid[:])
        w_sb = const.tile([P, cfg.totch], dt.bfloat16)
        nc.sync.dma_start(w_sb[:], wgt[:])
        iota_sb = const.tile([P, CPB * P], dt.bfloat16)
        nc.sync.dma_start(iota_sb[:], iota[:])
        w1_sb = const.tile([P, 2 * F_HID], dt.bfloat16)
        nc.sync.dma_start(w1_sb[:, 0:F_HID], w1[0])
        nc.sync.dma_start(w1_sb[:, F_HID:2 * F_HID], w1[1])
        w2_sb = const.tile([P, 2 * F_OUT], dt.bfloat16)
        nc.sync.dma_start(w2_sb[:, 0:F_OUT], w2[0])
        nc.sync.dma_start(w2_sb[:, F_OUT:2 * F_OUT], w2[1])
        b1_sb = const.tile([P, 2], dt.float32)
        nc.sync.dma_start(b1_sb[:, 0:1], b1[0])
        nc.sync.dma_start(b1_sb[:, 1:2], b1[1])
        b2_sb = const.tile([P, 1], dt.float32)
        nc.sync.dma_start(b2_sb[:], b2[:])

        zdt = dt.bfloat16 if os.environ.get("NO_FP8") else dt.float8e4
        zin = dram.tile([cfg.bp, F_IN], zdt)
        zall = dram.tile([cfg.ntot, F_IN], zdt, addr_space="Shared")
        yin = dram.tile([cfg.bp, F_OUT], dt.bfloat16)
        yall = dram.tile([cfg.ntot, F_OUT], dt.bfloat16, addr_space="Shared")

        # ------- phase Z: Z = feat @ W1 for own shard -------
        with tc.tile_pool(name="zpool", bufs=3) as zpool, \
             tc.tile_pool(name="ftpool", bufs=1) as ftpool, \
             tc.tile_pool(name="psum_z", bufs=2, space="PSUM") as psum_z:
            ft_sb = ftpool.tile([P, 2 * cfg.bp], dt.bfloat16)
            nc.sync.dma_start(ft_sb[:, 0:cfg.bp], featT[0])
            nc.sync.dma_start(ft_sb[:, cfg.bp:2 * cfg.bp], featT[1])
            for g in range(BL):
                pz = psum_z.tile([P, F_HID], dt.float32, space="PSUM", tag="pz")
                nc.tensor.matmul(
                    out=pz[:], lhsT=ft_sb[:, g * P:(g + 1) * P],
                    rhs=w1_sb[:, 0:F_HID], start=True, stop=False)
                nc.tensor.matmul(
                    out=pz[:], lhsT=ft_sb[:, cfg.bp + g * P:cfg.bp + (g + 1) * P],
                    rhs=w1_sb[:, F_HID:2 * F_HID], start=False, stop=True)
                zb = zpool.tile([P, F_HID], zdt, tag="zb")
                nc.vector.tensor_copy(zb[:], pz[:])
                nc.sync.dma_start(zin[g * P:(g + 1) * P, :], zb[:])

        if cfg.ncores > 1 and not os.environ.get("SKIP_COLL"):
            nc.gpsimd.collective_compute(
                "AllGather", mybir.AluOpType.bypass, replica_groups=rg,
                ins=[zin.opt()], outs=[zall.opt()])
        else:
            nc.sync.dma_start(zall[0:cfg.bp, :], zin[:])

        # ------- layer 1 + Y -------
        def layer(src, felem, gdt, nhalves, out_cb):
            active = {}  # block -> (S, ph list)

            def start_block(b):
                # S in transposed layout: S[p, s*CPB + c] = w[p,c]*(slot==s).
                # All operands keep a contiguous last dim -> DVE 2x mode.
                eq = spool.tile([P, CPB * P], dt.bfloat16, tag="eq", name="eq")
                S = spool.tile([P, CPB * P], dt.bfloat16, tag="S", name="S")
                if os.environ.get("SKIP_SGEN"):
                    nc.vector.tensor_copy(S[:], iota_sb[:, 0:CPB * P])
                    active[b] = (S, [psum_h.tile(
                        [P, P], dt.float32, space="PSUM", tag=f"ph{h}",
                        name=f"ph{h}") for h in range(nhalves)])
                    return
                nc.vector.tensor_tensor(
                    out=eq[:], in0=iota_sb[:],
                    in1=outer_bcast(seg_sb[:, b * CPB:(b + 1) * CPB], P),
                    op=mybir.AluOpType.is_equal)
                nc.vector.tensor_tensor(
                    out=S[:], in0=eq[:],
                    in1=outer_bcast(w_sb[:, b * CPB:(b + 1) * CPB], P),
                    op=mybir.AluOpType.mult)
                ph = [psum_h.tile([P, P], dt.float32, space="PSUM",
                                  tag=f"ph{h}", name=f"ph{h}")
                      for h in range(nhalves)]
                active[b] = (S, ph)

            for t in range(len(call_start)):
                c0, nch = int(call_start[t]), int(call_len[t])
                gt = gpool.tile([P, cfg.max_gcall, felem], gdt,
                                tag="gt", name="gt")
                win = src[int(base[t]):int(base[t] + width[t]), :]
                if os.environ.get("GATHER_HALF") and felem == 256:
                    # same calls/descriptors, half the bytes (junk numerics)
                    gh = gpool.tile([P, cfg.max_gcall, felem // 2],
                                    dt.bfloat16, tag="gh", name="gh")
                    nc.gpsimd.dma_gather(
                        gh[:, :nch, :], win, idx_sb[:, c0 * 8:(c0 + nch) * 8],
                        nch * P, nch * P, felem // 2, elem_step=felem,
                        queue_num=(t % NQ))
                    nc.vector.tensor_copy(gt[:, 0, 0:P], iota_sb[:, 0:P])
                else:
                    nc.gpsimd.dma_gather(
                        gt[:, :nch, :], win, idx_sb[:, c0 * 8:(c0 + nch) * 8],
                        nch * P, nch * P, felem, elem_step=felem,
                        queue_num=(t % NQ))
                for cc in range(nch):
                    c = c0 + cc
                    b, cloc = c // CPB, c % CPB
                    if cloc == 0:
                        start_block(b)
                    S, ph = active[b]
                    for h in range(nhalves):
                        nc.tensor.matmul(
                            out=ph[h][:],
                            lhsT=gt[:, cc, h * P:(h + 1) * P],
                            rhs=strided_cols(S[:], cloc, CPB, P),
                            start=(cloc == 0), stop=(cloc == CPB - 1))
                    if cloc == CPB - 1:
                        out_cb(b, ph)
                        del active[b]

        def l1_out(b, ph):
            hh = []
            for h in range(2):
                ht = hpool.tile([P, P], dt.bfloat16, tag=f"ht{h}")
                nc.scalar.activation(
                    ht[:], ph[h][:], mybir.ActivationFunctionType.Relu,
                    bias=b1_sb[:, h:h + 1])
                hh.append(ht)
            py = psum_y.tile([P, F_OUT], dt.float32, space="PSUM", tag="py")
            nc.tensor.matmul(out=py[:], lhsT=hh[0][:], rhs=w2_sb[:, 0:F_OUT],
                             start=True, stop=False)
            nc.tensor.matmul(out=py[:], lhsT=hh[1][:],
                             rhs=w2_sb[:, F_OUT:2 * F_OUT],
                             start=False, stop=True)
            yb = hpool.tile([P, F_OUT], dt.bfloat16, tag="yb")
            nc.vector.tensor_copy(yb[:], py[:])
            nc.sync.dma_start(yin[b * P:(b + 1) * P, :], yb[:])

        layer(zall, F_IN, zdt, 2, l1_out)

        if cfg.ncores > 1 and not os.environ.get("SKIP_COLL"):
            nc.gpsimd.collective_compute(
                "AllGather", mybir.AluOpType.bypass, replica_groups=rg,
                ins=[yin.opt()], outs=[yall.opt()])
        else:
            nc.sync.dma_start(yall[0:cfg.bp, :], yin[:])

        # ------- layer 2 -------
        def l2_out(b, ph):
            ob = hpool.tile([P, P], dt.float32, tag="ob")
            nc.scalar.activation(
                ob[:], ph[0][:], mybir.ActivationFunctionType.Identity,
                bias=b2_sb[:, 0:1])
            nc.sync.dma_start(outT[b], ob[:])

        layer(yall, F_OUT, dt.bfloat16, 1, l2_out)


# --------------------------------------------------------------------------
# Top level
# --------------------------------------------------------------------------

def declare_io(nc, cfg):
    dt = mybir.dt
    def di(name, shape, d):
        return nc.dram_tensor(name, shape, d, kind="ExternalInput").ap()
    ins = {
        "featT": di("featT", [2, P, cfg.bp], dt.bfloat16),
        "w1": di("w1", [2, P, F_HID], dt.bfloat16),
        "w2": di("w2", [2, P, F_OUT], dt.bfloat16),
        "b1": di("b1", [2, P, 1], dt.float32),
        "b2": di("b2", [P, 1], dt.float32),
        "iota": di("iota", [P, cfg.cpb * P], dt.bfloat16),
        "idxs": di("idxs", [P, cfg.totch * 8], dt.int16),
        "segid": di("segid", [P, cfg.totch], dt.bfloat16),
        "wgt": di("wgt", [P, cfg.totch], dt.bfloat16),
    }
    outs = {
        "outT": nc.dram_tensor("outT", [cfg.blocks, F_OUT, P], dt.float32,
                               kind="ExternalOutput").ap(),
    }
    return ins, outs


def build_nc(cfg, meta, repeat=1):
    nc = bacc.Bacc("TRN2", target_bir_lowering=False, debug=False,
                   num_devices=cfg.ncores, num_swdge_queues=NQ)
    ins, outs = declare_io(nc, cfg)
    with tile.TileContext(nc) as tc:
        for _ in range(repeat):
            build_program(tc, cfg, meta, outs, ins)
    nc.compile()
    return nc


# --------------------------------------------------------------------------
# Harness entry point: kernel(**inputs) with FULL unsharded inputs
# --------------------------------------------------------------------------

def kernel(feat, row, col, edge_weight, W1, b1, W2, b2):
    feat = np.asarray(feat, dtype=np.float32)
    row = np.asarray(row, dtype=np.int32)
    col = np.asarray(col, dtype=np.int32)
    edge_weight = np.asarray(edge_weight, dtype=np.float32)
    W1 = np.asarray(W1, dtype=np.float32)
    b1 = np.asarray(b1, dtype=np.float32)
    W2 = np.asarray(W2, dtype=np.float32)
    b2 = np.asarray(b2, dtype=np.float32)

    cfg = full_cfg()
    assert feat.shape == (cfg.n, F_IN) and row.shape == (cfg.e,)

    in_maps, meta = preprocess(cfg, feat, row, col, edge_weight, W1, b1, W2, b2)
    nc = build_nc(cfg, meta)

    from concourse.bass_utils import run_bass_kernel_spmd
    res = run_bass_kernel_spmd(nc, in_maps, core_ids=list(range(cfg.ncores)))
    outs = [{"outT": r["outT"]} for r in res.results]
    return assemble(cfg, meta, outs)

